# revision 48
# baseline (speedup 1.0000x reference)
"""Complex-valued relative-position attention (nn_CAttention) on 8 TRN2 cores.

Sharding: batch (4) x head-half (2) -> 8 cores. Each core computes its
batch's projections for its 4 heads, full attention for those heads, and a
row-split partial output projection. Host sums the two partial outputs per
batch and restacks.

v2 redesign (engine rebalance, f16 matmul inputs):
  - All matmul moving operands f16 (moving-side dtype sets the PE rate:
    1 cyc/row at any N; baseline's f32r qrel tail chunks ran at 4 cyc/row).
  - Half-inject: the i-part skew lands in the dots_i PSUM group via an
    extra identity-matmul accumulation (PE), so stage-B DVE work is two
    fused custom ops: er=(skw_r+dots_r)^2 (ADDSQ) and m2=er+dots_i^2
    (SQACC), instead of 2x ADDSQ + add.
  - qrel r/i parts pipeline through ONE f32 PSUM slot on alternating
    steps (r at step s, i at step s+1), with the PSUM->SBUF f16 copies
    split: r on ACT, i on DVE, 128-wide tails on Pool.
  - One combined skew DMA write [128,2304] and one 3-level-AP read
    [128,2048] per (h,I), both on the SP HWDGE queue (engine-free).
  - to_out negation folded into host-packed weights ([wo_r; -wo_i] /
    [wo_i; wo_r] row stacks): stage_D needs only 2 PSUM->OT copies (Pool).
  - attn transpose copy (atT) and V copies on Pool; ln/exp/exp softmax
    chain on ACT (pinned ln+exp table set, zero table switches).
"""
import functools
import numpy as np

import concourse.bass as bass
import concourse.bacc as bacc
import concourse.mybir as mybir
import concourse.tile as tile
from concourse.bass_utils import run_bass_kernel_spmd
from concourse.masks import make_identity

F32 = mybir.dt.float32
F16 = mybir.dt.float16
AF = mybir.ActivationFunctionType
ALU = mybir.AluOpType
U16 = mybir.dt.uint16

HEADS, DH, MAX_POS = 8, 64, 512
B, N, DIM = 4, 1024, 512
HPC = 4            # heads per core
KT = 4             # dim k-tiles (512/128)
NT = 8             # n tiles (1024/128)
SCALE = DH ** (-0.5)
PB = 3             # qrel->stage_B pipeline distance (iterations)


def register_custom():
    from concourse import dve_ops
    from concourse.dve_spec import Spec, Src0, Src1, AluOp, Bin, lower, sq
    from concourse.dve_spec import C0, C1
    from concourse.dve_uop import DveOpSpec

    def mk(name, body, ref):
        for op in dve_ops.OPS:
            if op.name == name:
                return op
        spec = Spec(body=body, reference=ref)
        opcode = dve_ops._CUSTOM_DVE_ROW_BASE + len(dve_ops.OPS)
        shas = {}
        for ver in ("v3",):
            s = DveOpSpec(name=name, opcode=opcode,
                          uops=lower(spec, ver=ver), rd1_en=True)
            shas[ver] = s.sha(ver)
        op = dve_ops.DveOp(name, spec, subdim=False, uops_sha=shas)
        dve_ops._SUB_OPCODE_FOR_NAME[op.name] = opcode
        dve_ops.OPS.append(op)
        dve_ops.CUSTOM_DVE_SPECS[op.name] = op.spec
        return op

    addsq = mk(
        "ADDSQ_ANT",
        sq(Bin(AluOp.ADD, Src0, Src1)),
        lambda in0, in1, s0, s1, imm2: (
            (in0.astype(np.float32) + in1.astype(np.float32)) ** 2),
    )
    sqacc = mk(
        "SQACC_ANT",
        Bin(AluOp.ADD, Src0, sq(Src1)),
        lambda in0, in1, s0, s1, imm2: (
            in0.astype(np.float32) + in1.astype(np.float32) ** 2),
    )
    # m = x*r0*(C0 - C1*(x*r0)*r0): one Newton-Raphson rsqrt step folded
    # with the final multiply; Src0 = x (=m2), Src1 = r0 (bit-trick rsqrt
    # seed), C0 = 1.5, C1 = 0.5. Produces sqrt(x) to ~0.2%.
    _t = Bin(AluOp.MULTIPLY, Src0, Src1)
    nsqrt = mk(
        "NSQRT_ANT",
        Bin(AluOp.MULTIPLY, _t,
            Bin(AluOp.SUBTRACT, C0,
                Bin(AluOp.MULTIPLY, C1,
                    Bin(AluOp.MULTIPLY, _t, Src1)))),
        lambda in0, in1, s0, s1, imm2: (
            (in0.astype(np.float32) * in1.astype(np.float32))
            * (s0 - s1 * (in0.astype(np.float32) * in1.astype(np.float32))
               * in1.astype(np.float32))),
    )
    return addsq, sqacc, nsqrt


def c_lo(i_blk):
    return 896 - 128 * i_blk


@functools.cache
def build_module():
    import concourse.tile_utils as tile_utils
    if getattr(tile_utils, "max_sbuf_usage", 0) < 208 * 1024:
        tile_utils.max_sbuf_usage = 208 * 1024

    # Pin the ACT engine to the ln+exp table set: every activation this
    # kernel emits (Ln, Exp, Copy/Identity) lives in that one set, so the
    # greedy table-load pass emits exactly one LoadActFuncSet.
    import concourse.bacc as bacc_mod
    if not getattr(bacc_mod, "_ant_act_tables_pinned", False):
        orig_gat = bacc_mod.get_activation_tables

        def pinned_gat(arch):
            full = orig_gat(arch)
            out = {}
            for name, funcs in full.items():
                if name != "natural_log_exp_and_others":
                    funcs = funcs - {mybir.ActivationFunctionType.Ln,
                                     mybir.ActivationFunctionType.Exp}
                out[name] = funcs
            return out

        bacc_mod.get_activation_tables = pinned_gat
        bacc_mod._ant_act_tables_pinned = True

    addsq, sqacc, nsqrt = register_custom()
    nc = bacc.Bacc("TRN2", target_bir_lowering=False, debug=False,
                   num_devices=8)

    din = {}
    for nm, shape, dt_ in [
        ("xt_r", [DIM, N], F16), ("xt_i", [DIM, N], F16),
        ("wq_a", [DIM, 512], F16), ("wq_b", [DIM, 512], F16),
        ("wk_a", [DIM, 512], F16), ("wk_b", [DIM, 512], F16),
        ("wv_a", [DIM, 512], F16), ("wv_b", [DIM, 512], F16),
        ("wo_sr", [DIM, 512], F16), ("wo_si", [DIM, 512], F16),
        ("rel_r", [128, 2048], F16), ("rel_i", [128, 2048], F16),
        ("bo_rt", [128, 4], F32), ("bo_it", [128, 4], F32),
        ("smask", [128, 1], F32),
    ]:
        din[nm] = nc.dram_tensor(nm, shape, dt_, kind="ExternalInput")
    o_r = nc.dram_tensor("o_r", [DIM, N], F32, kind="ExternalOutput")
    o_i = nc.dram_tensor("o_i", [DIM, N], F32, kind="ExternalOutput")

    with tile.TileContext(nc) as tc:
        with (
            tc.tile_pool(name="const", bufs=1) as cpool,
            tc.tile_pool(name="wts", bufs=6) as pwt,      # 4KB (weights)
            tc.tile_pool(name="xts", bufs=8) as pxt,      # 2KB (x tiles)
            tc.tile_pool(name="work", bufs=30) as pw,     # 2KB slots
            tc.tile_pool(name="qev", bufs=4) as pqe,      # 4.5KB slots
            tc.tile_pool(name="skew", bufs=6) as psk,     # 4KB slots
            tc.tile_pool(name="stacks", bufs=16) as pstk,  # 2KB slots
            tc.tile_pool(name="vstk", bufs=2) as pvp,     # 4KB slots
            tc.tile_pool(name="outsb", bufs=2) as pout,
            tc.tile_pool(name="small", bufs=16) as psm,
            tc.tile_pool(name="psD", bufs=3, space="PSUM") as psD,   # 2-bank
            tc.tile_pool(name="psC", bufs=2, space="PSUM") as psC,   # 1-bank
            tc.tile_pool(name="dram", bufs=7, space="DRAM") as pdram,
        ):
            # ---------------- constants ----------------
            id16 = cpool.tile([128, 128], F16, tag="id16")
            make_identity(nc, id16[:])
            smask = cpool.tile([128, 1], F32, tag="smask")
            nc.sync.dma_start(smask[:], din["smask"][:, :])

            def load_w4(nm):
                # [512, 512] dram -> [128, 4, 512] tile (one DMA)
                t = pwt.tile([128, 4, 512], F16, tag="w4", name=nm)
                nc.sync.dma_start(
                    t[:],
                    bass.AP(din[nm], 0, [[512, 128], [128 * 512, 4], [1, 512]]))
                return t


            # ---------------- phase P: projections ----------------
            A = [None] * HPC
            A2 = [None] * HPC
            Knat = [None] * HPC

            wq = (load_w4("wq_a"), load_w4("wq_b"))

            # xt tiles: [128, 1024] per (r/i, kt)
            xt = {}
            for nm in ("xt_r", "xt_i"):
                for kt in range(KT):
                    t = pxt.tile([128, 1024], F16, tag="xt",
                                 name=f"{nm}_{kt}")
                    nc.sync.dma_start(
                        t[:], bass.AP(din[nm], kt * 128 * N,
                                      [[N, 128], [1, 1024]]))
                    xt[(nm, kt)] = t

            wk = (load_w4("wk_a"), load_w4("wk_b"))
            wva = load_w4("wv_a")
            wvb = load_w4("wv_b")

            rel_r = cpool.tile([128, 2048], F16, tag="rel_r")
            rel_i = cpool.tile([128, 2048], F16, tag="rel_i")
            nc.sync.dma_start(rel_r[:], din["rel_r"][:, :])
            nc.sync.dma_start(rel_i[:], din["rel_i"][:, :])
            bo_rt = cpool.tile([128, 4], F32, tag="bo_rt")
            bo_it = cpool.tile([128, 4], F32, tag="bo_it")
            nc.sync.dma_start(bo_rt[:], din["bo_rt"][:, :])
            nc.sync.dma_start(bo_it[:], din["bo_it"][:, :])
            wo_sr = cpool.tile([128, 4, 512], F16, tag="wo_sr")
            wo_si = cpool.tile([128, 4, 512], F16, tag="wo_si")
            nc.sync.dma_start(
                wo_sr[:],
                bass.AP(din["wo_sr"], 0, [[512, 128], [128 * 512, 4], [1, 512]]))
            nc.sync.dma_start(
                wo_si[:],
                bass.AP(din["wo_si"], 0, [[512, 128], [128 * 512, 4], [1, 512]]))

            def emit_proj(kind, h, nh):
                wa, wb = wq if kind == "q" else wk
                hs = slice(h * 128, (h + 1) * 128)
                if nh == 0:
                    if kind == "q":
                        A[h] = pstk.tile([128, 1024], F16, tag="stk",
                                         name=f"A{h}")
                    else:
                        Knat[h] = pstk.tile([128, 1024], F16, tag="stk",
                                            name=f"Knat{h}")
                ns = slice(nh * 512, (nh + 1) * 512)
                ps = psD.tile([128, 512], F32, tag="pd",
                              name=f"ps{kind}_{h}_{nh}")
                for kt in range(KT):
                    nc.tensor.matmul(ps[:], wa[:, kt, hs],
                                     xt[("xt_r", kt)][:, ns],
                                     start=(kt == 0), stop=False)
                for kt in range(KT):
                    nc.tensor.matmul(ps[:], wb[:, kt, hs],
                                     xt[("xt_i", kt)][:, ns],
                                     start=False, stop=(kt == KT - 1))
                if kind == "q":
                    nc.scalar.mul(A[h][:, ns], ps[:], smask[:])
                else:
                    nc.scalar.copy(Knat[h][:, ns], ps[:])
                if kind == "q" and nh == 1:
                    # A2 = [s*qi; s*qr] built from A = [s*qr; -s*qi] on
                    # Pool (SBUF->SBUF), so dots_i = A2^T @ Knat and the
                    # Kni2 stack is never materialized
                    A2[h] = pstk.tile([128, 1024], F16, tag="stk",
                                      name=f"A2_{h}")
                    nc.gpsimd.tensor_scalar_mul(A2[h][0:64, :],
                                                A[h][64:128, :], -1.0)
                    nc.gpsimd.tensor_copy(A2[h][64:128, :], A[h][0:64, :])

            for kind, h in (("q", 0), ("q", 1), ("k", 0)):
                for nh in range(2):
                    emit_proj(kind, h, nh)
            Vpp = [pvp.tile([128, 8, 256], F16, tag="vs", name=f"Vpp{p}")
                   for p in range(2)]

            def emit_vproj(p, J):
                # two heads per matmul (256-wide output)
                hs = slice(p * 256, (p + 1) * 256)
                js = slice(J * 128, (J + 1) * 128)
                vps = psC.tile([128, 256], F32, tag="pc",
                               name=f"vps_{p}_{J}")
                for kt in range(KT):
                    nc.tensor.matmul(vps[:],
                                     xt[("xt_r", kt)][:, js],
                                     wva[:, kt, hs],
                                     start=(kt == 0), stop=False)
                for kt in range(KT):
                    nc.tensor.matmul(vps[:],
                                     xt[("xt_i", kt)][:, js],
                                     wvb[:, kt, hs],
                                     start=False, stop=(kt == KT - 1))
                nc.scalar.copy(Vpp[p][:, J, :], vps[:])

            units = ([("k", 1, 0), ("k", 1, 1)]
                     + [("v", 0, J) for J in range(NT)]
                     + [(k, h, nh) for h in (2, 3) for k in ("q", "k")
                        for nh in range(2)]
                     + [("v", 1, J) for J in range(NT)])

            # OT stacks for the output projection:
            # OT[0] = or heads 0,1 | OT[1] = or heads 2,3
            # OT[2] = oi heads 0,1 | OT[3] = oi heads 2,3
            OT = [pstk.tile([128, 1024], F16, tag="stk", name=f"OT{t}")
                  for t in range(4)]

            # ---------------- phase A: pipelined attention ----------------
            # Per (h, I): qrel r-part at step s, i-part + skew write/read at
            # step s+1, dots + softmax at s+PB, transpose at s+PB+2, AV at
            # s+PB+3.
            def emit_qrel_r(h, I):
                isl = slice(I * 128, (I + 1) * 128)
                lo = c_lo(I)
                qe = pqe.tile([128, 2, 1152], F16, tag="qe",
                              name=f"qe_{h}_{I}")
                qtl = psC.tile([128, 2, 128], F32, tag="pc",
                               name=f"qtl_{h}_{I}")
                qps = psD.tile([128, 1024], F32, tag="pd",
                               name=f"qpsr_{h}_{I}")
                for c0 in (0, 512):
                    nc.tensor.matmul(qps[:, c0:c0 + 512], A[h][:, isl],
                                     rel_r[:, lo + c0:lo + c0 + 512],
                                     start=True, stop=True)
                nc.tensor.matmul(qtl[:, 0, :], A[h][:, isl],
                                 rel_r[:, lo + 1024:lo + 1152],
                                 start=True, stop=True)
                nc.scalar.copy(qe[:, 0, 0:1024], qps[:])
                return {"qe": qe, "qtl": qtl}

            def emit_qrel_i(h, I, st):
                isl = slice(I * 128, (I + 1) * 128)
                lo = c_lo(I)
                qe, qtl = st["qe"], st["qtl"]
                qps = psD.tile([128, 1024], F32, tag="pd",
                               name=f"qpsi_{h}_{I}")
                for c0 in (0, 512):
                    nc.tensor.matmul(qps[:, c0:c0 + 512], A[h][:, isl],
                                     rel_i[:, lo + c0:lo + c0 + 512],
                                     start=True, stop=True)
                nc.tensor.matmul(qtl[:, 1, :], A[h][:, isl],
                                 rel_i[:, lo + 1024:lo + 1152],
                                 start=True, stop=True)
                nc.scalar.copy(qe[:, 1, 0:1024], qps[:])
                nc.scalar.copy(qe[:, :, 1024:1152], qtl[:])
                slot = pdram.tile([128, 2304], F16, tag="qrev",
                                  name=f"qrev_{h}_{I}")
                nc.sync.dma_start(slot[:, :], qe[:])
                skw = psk.tile([128, 2048], F16, tag="skw",
                               name=f"skew_{h}_{I}")
                nc.sync.dma_start(
                    skw[:],
                    bass.AP(slot.tensor, 127,
                            [[2303, 128], [1152, 2], [1, 1024]]))
                return skw

            def stage_B(h, I, skw):
                isl = slice(I * 128, (I + 1) * 128)
                dpsr = psD.tile([128, 1024], F32, tag="pd",
                                name=f"dpsr_{h}_{I}")
                dpsi = psD.tile([128, 1024], F32, tag="pd",
                                name=f"dpsi_{h}_{I}")
                # i-part skew injected into the dots_i PSUM group via
                # identity-matmul accumulation (DVE can read only one PSUM
                # operand per op, so r-part adds via the fused ADDSQ)
                for nh in range(2):
                    ns = slice(nh * 512, (nh + 1) * 512)
                    nc.tensor.matmul(dpsr[:, ns], A[h][:, isl],
                                     Knat[h][:, ns], start=True, stop=True)
                    nc.tensor.matmul(dpsi[:, ns], A2[h][:, isl],
                                     Knat[h][:, ns], start=True, stop=False)
                    nc.tensor.matmul(dpsi[:, ns], id16[:],
                                     skw[:, 1024 + nh * 512:1024 + nh * 512 + 512],
                                     start=False, stop=True)
                er = pw.tile([128, 1024], F16, tag="wk", name=f"er_{h}_{I}")
                nc.vector._custom_dve(addsq, out=er[:],
                                      in0=skw[:, 0:1024], in1=dpsr[:])
                m2 = pw.tile([128, 1024], F16, tag="wk", name=f"m2_{h}_{I}")
                nc.vector._custom_dve(sqacc, out=m2[:],
                                      in0=er[:], in1=dpsi[:])
                # sqrt(m2) without the ACT ln/exp round trip: integer-view
                # rsqrt bit-seed on Pool, then one fused Newton step * m2 on
                # DVE (NSQRT, next pipeline step).  bits(r0) = 22971 -
                # bits(m2)/2.
                r0 = pw.tile([128, 1024], F16, tag="wk", name=f"r0_{h}_{I}")
                nc.gpsimd.tensor_scalar(
                    r0[:].bitcast(U16), m2[:].bitcast(U16),
                    -0.5, 22971.0, ALU.mult, ALU.add)
                return {"m2": m2, "r0": r0}

            def stage_B2(h, I, st):
                m2, r0 = st["m2"], st["r0"]
                mt = pw.tile([128, 1024], F16, tag="wk", name=f"mt_{h}_{I}")
                nc.vector._custom_dve(nsqrt, out=mt[:],
                                      in0=m2[:], in1=r0[:],
                                      s0=1.5, s1=0.5)
                attn = pw.tile([128, 1024], F16, tag="wk",
                               name=f"attn_{h}_{I}")
                rs = psm.tile([128, 1], F32, tag="sm", name=f"rs_{h}_{I}")
                nc.scalar.activation(attn[:], mt[:], AF.Exp, accum_out=rs[:])
                rc = psm.tile([128, 1], F32, tag="sm", name=f"rc_{h}_{I}")
                nc.vector.reciprocal(rc[:], rs[:])
                # normalize on Pool (SBUF-only op), a full step ahead of the
                # transposes that consume attn
                nc.gpsimd.tensor_scalar_mul(attn[:], attn[:], rc[:])
                return {"attn": attn}

            def stage_C(h, I, st):
                attn = st["attn"]
                tps = psC.tile([128, 1024], F16, tag="pc", name=f"tps_{h}_{I}")
                for J in range(NT):
                    js = slice(J * 128, (J + 1) * 128)
                    nc.tensor.transpose(tps[:, js], attn[:, js], id16[:])
                atT = pw.tile([128, 1024], F16, tag="wk", name=f"atT_{h}_{I}")
                nc.vector.tensor_copy(atT[:], tps[:])
                return atT

            def stage_D(h, I, atT):
                isl = slice(I * 128, (I + 1) * 128)
                avs = psC.tile([128, 128], F32, tag="pc", name=f"avs_{h}_{I}")
                vsl = slice((h % 2) * 128, (h % 2) * 128 + 128)
                for J in range(NT):
                    js = slice(J * 128, (J + 1) * 128)
                    nc.tensor.matmul(avs[:], Vpp[h // 2][:, J, vsl],
                                     atT[:, js],
                                     start=(J == 0), stop=(J == NT - 1))
                prt = slice((h % 2) * 64, (h % 2) * 64 + 64)
                nc.vector.tensor_copy(OT[h // 2][prt, isl], avs[0:64, :])
                nc.vector.tensor_copy(OT[2 + h // 2][prt, isl],
                                      avs[64:128, :])

            def emit_outproj(nh):
                ns = slice(nh * 512, (nh + 1) * 512)
                for part, wo_t, bo_t in (("r", wo_sr, bo_rt),
                                         ("i", wo_si, bo_it)):
                    for dt_ in range(4):
                        ds = slice(dt_ * 128, (dt_ + 1) * 128)
                        ops = psC.tile([128, 512], F32, tag="pc",
                                       name=f"ops_{part}_{dt_}_{nh}")
                        for j in range(4):
                            nc.tensor.matmul(ops[:], wo_t[:, j, ds],
                                             OT[j][:, ns],
                                             start=(j == 0), stop=(j == 3))
                        osb = pout.tile([128, 512], F32, tag="ot",
                                        name=f"osb_{part}_{dt_}_{nh}")
                        nc.scalar.activation(osb[:], ops[:], AF.Identity,
                                             bias=bo_t[:, dt_:dt_ + 1])
                        dst = o_r if part == "r" else o_i
                        nc.sync.dma_start(
                            bass.AP(dst, dt_ * 128 * N + nh * 512,
                                    [[N, 128], [1, 512]]),
                            osb[:])

            flat = [(h, I) for h in range(HPC) for I in range(NT)]
            PB2, PC, PD = PB + 1, PB + 2, PB + 3
            rmap, skewmap, bmap, b2map, cmap = {}, {}, {}, {}, {}
            # late stages are emitted FIRST within each step so that
            # dependency waits of the stage-B tail never head-of-line block
            # the in-order engine queues for already-runnable work
            for s in range(len(flat) + PD + 1):
                for _ in range(2):
                    if units:
                        u = units.pop(0)
                        if u[0] == "v":
                            emit_vproj(u[1], u[2])
                        else:
                            emit_proj(*u)
                if s < len(flat):
                    h, I = flat[s]
                    rmap[(h, I)] = emit_qrel_r(h, I)
                if 1 <= s < len(flat) + 1:
                    h, I = flat[s - 1]
                    skewmap[(h, I)] = emit_qrel_i(h, I, rmap.pop((h, I)))
                if PB <= s < len(flat) + PB:
                    h, I = flat[s - PB]
                    bmap[(h, I)] = stage_B(h, I, skewmap.pop((h, I)))
                if PB2 <= s < len(flat) + PB2:
                    h, I = flat[s - PB2]
                    b2map[(h, I)] = stage_B2(h, I, bmap.pop((h, I)))
                if PC <= s < len(flat) + PC:
                    h, I = flat[s - PC]
                    cmap[(h, I)] = stage_C(h, I, b2map.pop((h, I)))
                if PD <= s < len(flat) + PD:
                    h, I = flat[s - PD]
                    stage_D(h, I, cmap.pop((h, I)))
                    if (h, I) == (HPC - 1, 3):
                        emit_outproj(0)
            emit_outproj(1)

    nc.compile()
    return nc, addsq


def _prep_core_inputs(inputs, core):
    b, half = core // 2, core % 2
    x = inputs["x"]
    f16 = np.float16
    f32 = np.float32
    xt_r = np.ascontiguousarray(x[b, :, :, 0].T).astype(f16)
    xt_i = np.ascontiguousarray(x[b, :, :, 1].T).astype(f16)

    def pack_ab(wr, wi):
        a = np.empty((DIM, 512), f32)
        bb = np.empty((DIM, 512), f32)
        for hl in range(HPC):
            gh = half * HPC + hl
            cs = slice(gh * DH, (gh + 1) * DH)
            a[:, hl * 128:hl * 128 + 64] = wr[:, cs]
            a[:, hl * 128 + 64:hl * 128 + 128] = wi[:, cs]
            bb[:, hl * 128:hl * 128 + 64] = -wi[:, cs]
            bb[:, hl * 128 + 64:hl * 128 + 128] = wr[:, cs]
        return a.astype(f16), bb.astype(f16)

    wq_a, wq_b = pack_ab(inputs["wq_r"], inputs["wq_i"])
    wk_a, wk_b = pack_ab(inputs["wkv_r"][:, :512], inputs["wkv_i"][:, :512])
    wv_a, wv_b = pack_ab(inputs["wkv_r"][:, 512:], inputs["wkv_i"][:, 512:])

    rs = slice(half * 256, (half + 1) * 256)
    wo_sr = np.concatenate(
        [inputs["wo_r"][rs, :], -inputs["wo_i"][rs, :]], 0).astype(f16)
    wo_si = np.concatenate(
        [inputs["wo_i"][rs, :], inputs["wo_r"][rs, :]], 0).astype(f16)

    e = np.arange(2047)
    t_ext = inputs["rel_emb"][np.clip(e - 1023, -MAX_POS, MAX_POS) + MAX_POS]
    relrev = t_ext[::-1].astype(f32)           # [2047, 64]
    rel_r = np.zeros((128, 2048), f32)
    rel_i = np.zeros((128, 2048), f32)
    rel_r[0:64, 0:2047] = relrev.T
    rel_i[64:128, 0:2047] = -relrev.T

    bscale = 1.0 if half == 0 else 0.0
    bo_rt = np.ascontiguousarray(
        inputs["bo_r"].reshape(4, 128).T * bscale).astype(f32)
    bo_it = np.ascontiguousarray(
        inputs["bo_i"].reshape(4, 128).T * bscale).astype(f32)
    smask = np.concatenate(
        [np.full(64, SCALE, f32), np.full(64, -SCALE, f32)]).reshape(128, 1)

    return {
        "xt_r": xt_r, "xt_i": xt_i,
        "wq_a": wq_a, "wq_b": wq_b, "wk_a": wk_a, "wk_b": wk_b,
        "wv_a": wv_a, "wv_b": wv_b,
        "wo_sr": wo_sr, "wo_si": wo_si,
        "rel_r": rel_r.astype(f16), "rel_i": rel_i.astype(f16),
        "bo_rt": bo_rt, "bo_it": bo_it, "smask": smask,
    }


_last_results = {}


def kernel(**inputs):
    inputs = {k: np.asarray(v) for k, v in inputs.items()}
    nc, _ = build_module()
    in_maps = [_prep_core_inputs(inputs, c) for c in range(8)]
    res = run_bass_kernel_spmd(nc, in_maps, core_ids=list(range(8)))
    _last_results["res"] = res

    out = np.empty((B, N, DIM, 2), np.float32)
    for b in range(B):
        r = res.results[2 * b]["o_r"] + res.results[2 * b + 1]["o_r"]
        i = res.results[2 * b]["o_i"] + res.results[2 * b + 1]["o_i"]
        out[b, :, :, 0] = r.T
        out[b, :, :, 1] = i.T
    return out


# revision 49
# speedup vs baseline: 1.0003x; 1.0003x over previous
"""Complex-valued relative-position attention (nn_CAttention) on 8 TRN2 cores.

Sharding: batch (4) x head-half (2) -> 8 cores. Each core computes its
batch's projections for its 4 heads, full attention for those heads, and a
row-split partial output projection. Host sums the two partial outputs per
batch and restacks.

v2 redesign (engine rebalance, f16 matmul inputs):
  - All matmul moving operands f16 (moving-side dtype sets the PE rate:
    1 cyc/row at any N; baseline's f32r qrel tail chunks ran at 4 cyc/row).
  - Half-inject: the i-part skew lands in the dots_i PSUM group via an
    extra identity-matmul accumulation (PE), so stage-B DVE work is two
    fused custom ops: er=(skw_r+dots_r)^2 (ADDSQ) and m2=er+dots_i^2
    (SQACC), instead of 2x ADDSQ + add.
  - qrel r/i parts pipeline through ONE f32 PSUM slot on alternating
    steps (r at step s, i at step s+1), with the PSUM->SBUF f16 copies
    split: r on ACT, i on DVE, 128-wide tails on Pool.
  - One combined skew DMA write [128,2304] and one 3-level-AP read
    [128,2048] per (h,I), both on the SP HWDGE queue (engine-free).
  - to_out negation folded into host-packed weights ([wo_r; -wo_i] /
    [wo_i; wo_r] row stacks): stage_D needs only 2 PSUM->OT copies (Pool).
  - attn transpose copy (atT) and V copies on Pool; ln/exp/exp softmax
    chain on ACT (pinned ln+exp table set, zero table switches).
"""
import functools
import numpy as np

import concourse.bass as bass
import concourse.bacc as bacc
import concourse.mybir as mybir
import concourse.tile as tile
from concourse.bass_utils import run_bass_kernel_spmd
from concourse.masks import make_identity

F32 = mybir.dt.float32
F16 = mybir.dt.float16
AF = mybir.ActivationFunctionType
ALU = mybir.AluOpType
U16 = mybir.dt.uint16

HEADS, DH, MAX_POS = 8, 64, 512
B, N, DIM = 4, 1024, 512
HPC = 4            # heads per core
KT = 4             # dim k-tiles (512/128)
NT = 8             # n tiles (1024/128)
SCALE = DH ** (-0.5)
PB = 3             # qrel->stage_B pipeline distance (iterations)


def register_custom():
    from concourse import dve_ops
    from concourse.dve_spec import Spec, Src0, Src1, AluOp, Bin, lower, sq
    from concourse.dve_spec import C0, C1
    from concourse.dve_uop import DveOpSpec

    def mk(name, body, ref):
        for op in dve_ops.OPS:
            if op.name == name:
                return op
        spec = Spec(body=body, reference=ref)
        opcode = dve_ops._CUSTOM_DVE_ROW_BASE + len(dve_ops.OPS)
        shas = {}
        for ver in ("v3",):
            s = DveOpSpec(name=name, opcode=opcode,
                          uops=lower(spec, ver=ver), rd1_en=True)
            shas[ver] = s.sha(ver)
        op = dve_ops.DveOp(name, spec, subdim=False, uops_sha=shas)
        dve_ops._SUB_OPCODE_FOR_NAME[op.name] = opcode
        dve_ops.OPS.append(op)
        dve_ops.CUSTOM_DVE_SPECS[op.name] = op.spec
        return op

    addsq = mk(
        "ADDSQ_ANT",
        sq(Bin(AluOp.ADD, Src0, Src1)),
        lambda in0, in1, s0, s1, imm2: (
            (in0.astype(np.float32) + in1.astype(np.float32)) ** 2),
    )
    sqacc = mk(
        "SQACC_ANT",
        Bin(AluOp.ADD, Src0, sq(Src1)),
        lambda in0, in1, s0, s1, imm2: (
            in0.astype(np.float32) + in1.astype(np.float32) ** 2),
    )
    # m = x*r0*(C0 - C1*(x*r0)*r0): one Newton-Raphson rsqrt step folded
    # with the final multiply; Src0 = x (=m2), Src1 = r0 (bit-trick rsqrt
    # seed), C0 = 1.5, C1 = 0.5. Produces sqrt(x) to ~0.2%.
    _t = Bin(AluOp.MULTIPLY, Src0, Src1)
    nsqrt = mk(
        "NSQRT_ANT",
        Bin(AluOp.MULTIPLY, _t,
            Bin(AluOp.SUBTRACT, C0,
                Bin(AluOp.MULTIPLY, C1,
                    Bin(AluOp.MULTIPLY, _t, Src1)))),
        lambda in0, in1, s0, s1, imm2: (
            (in0.astype(np.float32) * in1.astype(np.float32))
            * (s0 - s1 * (in0.astype(np.float32) * in1.astype(np.float32))
               * in1.astype(np.float32))),
    )
    return addsq, sqacc, nsqrt


def c_lo(i_blk):
    return 896 - 128 * i_blk


@functools.cache
def build_module():
    import concourse.tile_utils as tile_utils
    if getattr(tile_utils, "max_sbuf_usage", 0) < 208 * 1024:
        tile_utils.max_sbuf_usage = 208 * 1024

    # Pin the ACT engine to the ln+exp table set: every activation this
    # kernel emits (Ln, Exp, Copy/Identity) lives in that one set, so the
    # greedy table-load pass emits exactly one LoadActFuncSet.
    import concourse.bacc as bacc_mod
    if not getattr(bacc_mod, "_ant_act_tables_pinned", False):
        orig_gat = bacc_mod.get_activation_tables

        def pinned_gat(arch):
            full = orig_gat(arch)
            out = {}
            for name, funcs in full.items():
                if name != "natural_log_exp_and_others":
                    funcs = funcs - {mybir.ActivationFunctionType.Ln,
                                     mybir.ActivationFunctionType.Exp}
                out[name] = funcs
            return out

        bacc_mod.get_activation_tables = pinned_gat
        bacc_mod._ant_act_tables_pinned = True

    addsq, sqacc, nsqrt = register_custom()
    nc = bacc.Bacc("TRN2", target_bir_lowering=False, debug=False,
                   num_devices=8)

    din = {}
    for nm, shape, dt_ in [
        ("xt_r", [DIM, N], F16), ("xt_i", [DIM, N], F16),
        ("wq_a", [DIM, 512], F16), ("wq_b", [DIM, 512], F16),
        ("wk_a", [DIM, 512], F16), ("wk_b", [DIM, 512], F16),
        ("wv_a", [DIM, 512], F16), ("wv_b", [DIM, 512], F16),
        ("wo_sr", [DIM, 512], F16), ("wo_si", [DIM, 512], F16),
        ("rel_r", [128, 2048], F16), ("rel_i", [128, 2048], F16),
        ("bo_rt", [128, 4], F32), ("bo_it", [128, 4], F32),
        ("smask", [128, 1], F32),
    ]:
        din[nm] = nc.dram_tensor(nm, shape, dt_, kind="ExternalInput")
    o_r = nc.dram_tensor("o_r", [DIM, N], F32, kind="ExternalOutput")
    o_i = nc.dram_tensor("o_i", [DIM, N], F32, kind="ExternalOutput")

    with tile.TileContext(nc) as tc:
        with (
            tc.tile_pool(name="const", bufs=1) as cpool,
            tc.tile_pool(name="wts", bufs=6) as pwt,      # 4KB (weights)
            tc.tile_pool(name="xts", bufs=8) as pxt,      # 2KB (x tiles)
            tc.tile_pool(name="work", bufs=30) as pw,     # 2KB slots
            tc.tile_pool(name="qev", bufs=4) as pqe,      # 4.5KB slots
            tc.tile_pool(name="skew", bufs=6) as psk,     # 4KB slots
            tc.tile_pool(name="stacks", bufs=16) as pstk,  # 2KB slots
            tc.tile_pool(name="vstk", bufs=2) as pvp,     # 4KB slots
            tc.tile_pool(name="outsb", bufs=2) as pout,
            tc.tile_pool(name="small", bufs=16) as psm,
            tc.tile_pool(name="psD", bufs=3, space="PSUM") as psD,   # 2-bank
            tc.tile_pool(name="psC", bufs=2, space="PSUM") as psC,   # 1-bank
            tc.tile_pool(name="dram", bufs=7, space="DRAM") as pdram,
        ):
            # ---------------- constants ----------------
            id16 = cpool.tile([128, 128], F16, tag="id16")
            make_identity(nc, id16[:])
            smask = cpool.tile([128, 1], F32, tag="smask")
            nc.sync.dma_start(smask[:], din["smask"][:, :])

            def load_w4(nm):
                # [512, 512] dram -> [128, 4, 512] tile (one DMA)
                t = pwt.tile([128, 4, 512], F16, tag="w4", name=nm)
                nc.sync.dma_start(
                    t[:],
                    bass.AP(din[nm], 0, [[512, 128], [128 * 512, 4], [1, 512]]))
                return t


            # ---------------- phase P: projections ----------------
            A = [None] * HPC
            A2 = [None] * HPC
            Knat = [None] * HPC

            wq = (load_w4("wq_a"), load_w4("wq_b"))

            # xt tiles: [128, 1024] per (r/i, kt)
            xt = {}
            for nm in ("xt_r", "xt_i"):
                for kt in range(KT):
                    t = pxt.tile([128, 1024], F16, tag="xt",
                                 name=f"{nm}_{kt}")
                    nc.sync.dma_start(
                        t[:], bass.AP(din[nm], kt * 128 * N,
                                      [[N, 128], [1, 1024]]))
                    xt[(nm, kt)] = t

            wk = (load_w4("wk_a"), load_w4("wk_b"))
            wva = load_w4("wv_a")
            wvb = load_w4("wv_b")

            rel_r = cpool.tile([128, 2048], F16, tag="rel_r")
            rel_i = cpool.tile([128, 2048], F16, tag="rel_i")
            nc.sync.dma_start(rel_r[:], din["rel_r"][:, :])
            nc.sync.dma_start(rel_i[:], din["rel_i"][:, :])
            bo_rt = cpool.tile([128, 4], F32, tag="bo_rt")
            bo_it = cpool.tile([128, 4], F32, tag="bo_it")
            nc.sync.dma_start(bo_rt[:], din["bo_rt"][:, :])
            nc.sync.dma_start(bo_it[:], din["bo_it"][:, :])
            wo_sr = cpool.tile([128, 4, 512], F16, tag="wo_sr")
            wo_si = cpool.tile([128, 4, 512], F16, tag="wo_si")
            nc.sync.dma_start(
                wo_sr[:],
                bass.AP(din["wo_sr"], 0, [[512, 128], [128 * 512, 4], [1, 512]]))
            nc.sync.dma_start(
                wo_si[:],
                bass.AP(din["wo_si"], 0, [[512, 128], [128 * 512, 4], [1, 512]]))

            def emit_proj(kind, h, nh):
                wa, wb = wq if kind == "q" else wk
                hs = slice(h * 128, (h + 1) * 128)
                if nh == 0:
                    if kind == "q":
                        A[h] = pstk.tile([128, 1024], F16, tag="stk",
                                         name=f"A{h}")
                    else:
                        Knat[h] = pstk.tile([128, 1024], F16, tag="stk",
                                            name=f"Knat{h}")
                ns = slice(nh * 512, (nh + 1) * 512)
                ps = psD.tile([128, 512], F32, tag="pd",
                              name=f"ps{kind}_{h}_{nh}")
                for kt in range(KT):
                    nc.tensor.matmul(ps[:], wa[:, kt, hs],
                                     xt[("xt_r", kt)][:, ns],
                                     start=(kt == 0), stop=False)
                for kt in range(KT):
                    nc.tensor.matmul(ps[:], wb[:, kt, hs],
                                     xt[("xt_i", kt)][:, ns],
                                     start=False, stop=(kt == KT - 1))
                if kind == "q":
                    nc.scalar.mul(A[h][:, ns], ps[:], smask[:])
                else:
                    nc.scalar.copy(Knat[h][:, ns], ps[:])
                if kind == "q" and nh == 1:
                    # A2 = [s*qi; s*qr] built from A = [s*qr; -s*qi] on
                    # Pool (SBUF->SBUF), so dots_i = A2^T @ Knat and the
                    # Kni2 stack is never materialized
                    A2[h] = pstk.tile([128, 1024], F16, tag="stk",
                                      name=f"A2_{h}")
                    nc.gpsimd.tensor_scalar_mul(A2[h][0:64, :],
                                                A[h][64:128, :], -1.0)
                    nc.gpsimd.tensor_copy(A2[h][64:128, :], A[h][0:64, :])

            for kind, h in (("q", 0), ("q", 1), ("k", 0)):
                for nh in range(2):
                    emit_proj(kind, h, nh)
            Vpp = [pvp.tile([128, 8, 256], F16, tag="vs", name=f"Vpp{p}")
                   for p in range(2)]

            def emit_vproj(p, J):
                # two heads per matmul (256-wide output)
                hs = slice(p * 256, (p + 1) * 256)
                js = slice(J * 128, (J + 1) * 128)
                vps = psC.tile([128, 256], F32, tag="pc",
                               name=f"vps_{p}_{J}")
                for kt in range(KT):
                    nc.tensor.matmul(vps[:],
                                     xt[("xt_r", kt)][:, js],
                                     wva[:, kt, hs],
                                     start=(kt == 0), stop=False)
                for kt in range(KT):
                    nc.tensor.matmul(vps[:],
                                     xt[("xt_i", kt)][:, js],
                                     wvb[:, kt, hs],
                                     start=False, stop=(kt == KT - 1))
                nc.scalar.copy(Vpp[p][:, J, :], vps[:])

            units = ([("k", 1, 0), ("k", 1, 1)]
                     + [("v", 0, J) for J in range(NT)]
                     + [(k, h, nh) for h in (2, 3) for k in ("q", "k")
                        for nh in range(2)]
                     + [("v", 1, J) for J in range(NT)])

            # OT stacks for the output projection:
            # OT[0] = or heads 0,1 | OT[1] = or heads 2,3
            # OT[2] = oi heads 0,1 | OT[3] = oi heads 2,3
            OT = [pstk.tile([128, 1024], F16, tag="stk", name=f"OT{t}")
                  for t in range(4)]

            # ---------------- phase A: pipelined attention ----------------
            # Per (h, I): qrel r-part at step s, i-part + skew write/read at
            # step s+1, dots + softmax at s+PB, transpose at s+PB+2, AV at
            # s+PB+3.
            def emit_qrel_r(h, I):
                isl = slice(I * 128, (I + 1) * 128)
                lo = c_lo(I)
                qe = pqe.tile([128, 2, 1152], F16, tag="qe",
                              name=f"qe_{h}_{I}")
                qtl = psC.tile([128, 2, 128], F32, tag="pc",
                               name=f"qtl_{h}_{I}")
                qps = psD.tile([128, 1024], F32, tag="pd",
                               name=f"qpsr_{h}_{I}")
                for c0 in (0, 512):
                    nc.tensor.matmul(qps[:, c0:c0 + 512], A[h][:, isl],
                                     rel_r[:, lo + c0:lo + c0 + 512],
                                     start=True, stop=True)
                nc.tensor.matmul(qtl[:, 0, :], A[h][:, isl],
                                 rel_r[:, lo + 1024:lo + 1152],
                                 start=True, stop=True)
                nc.scalar.copy(qe[:, 0, 0:1024], qps[:])
                return {"qe": qe, "qtl": qtl}

            def emit_qrel_i(h, I, st):
                isl = slice(I * 128, (I + 1) * 128)
                lo = c_lo(I)
                qe, qtl = st["qe"], st["qtl"]
                qps = psD.tile([128, 1024], F32, tag="pd",
                               name=f"qpsi_{h}_{I}")
                for c0 in (0, 512):
                    nc.tensor.matmul(qps[:, c0:c0 + 512], A[h][:, isl],
                                     rel_i[:, lo + c0:lo + c0 + 512],
                                     start=True, stop=True)
                nc.tensor.matmul(qtl[:, 1, :], A[h][:, isl],
                                 rel_i[:, lo + 1024:lo + 1152],
                                 start=True, stop=True)
                nc.scalar.copy(qe[:, 1, 0:1024], qps[:])
                nc.scalar.copy(qe[:, :, 1024:1152], qtl[:])
                slot = pdram.tile([128, 2304], F16, tag="qrev",
                                  name=f"qrev_{h}_{I}")
                nc.sync.dma_start(slot[:, :], qe[:])
                skw = psk.tile([128, 2048], F16, tag="skw",
                               name=f"skew_{h}_{I}")
                nc.sync.dma_start(
                    skw[:],
                    bass.AP(slot.tensor, 127,
                            [[2303, 128], [1152, 2], [1, 1024]]))
                return skw

            def stage_B(h, I, skw):
                isl = slice(I * 128, (I + 1) * 128)
                dpsr = psD.tile([128, 1024], F32, tag="pd",
                                name=f"dpsr_{h}_{I}")
                dpsi = psD.tile([128, 1024], F32, tag="pd",
                                name=f"dpsi_{h}_{I}")
                # i-part skew injected into the dots_i PSUM group via
                # identity-matmul accumulation (DVE can read only one PSUM
                # operand per op, so r-part adds via the fused ADDSQ)
                for nh in range(2):
                    ns = slice(nh * 512, (nh + 1) * 512)
                    nc.tensor.matmul(dpsr[:, ns], A[h][:, isl],
                                     Knat[h][:, ns], start=True, stop=True)
                    nc.tensor.matmul(dpsi[:, ns], A2[h][:, isl],
                                     Knat[h][:, ns], start=True, stop=False)
                    nc.tensor.matmul(dpsi[:, ns], id16[:],
                                     skw[:, 1024 + nh * 512:1024 + nh * 512 + 512],
                                     start=False, stop=True)
                er = pw.tile([128, 1024], F16, tag="wk", name=f"er_{h}_{I}")
                nc.vector._custom_dve(addsq, out=er[:],
                                      in0=skw[:, 0:1024], in1=dpsr[:])
                m2 = pw.tile([128, 1024], F16, tag="wk", name=f"m2_{h}_{I}")
                nc.vector._custom_dve(sqacc, out=m2[:],
                                      in0=er[:], in1=dpsi[:])
                # sqrt(m2) without the ACT ln/exp round trip: integer-view
                # rsqrt bit-seed on Pool, then one fused Newton step * m2 on
                # DVE (NSQRT, next pipeline step).  bits(r0) = 22971 -
                # bits(m2)/2.
                r0 = pw.tile([128, 1024], F16, tag="wk", name=f"r0_{h}_{I}")
                nc.gpsimd.tensor_scalar(
                    r0[:].bitcast(U16), m2[:].bitcast(U16),
                    -0.5, 22971.0, ALU.mult, ALU.add)
                return {"m2": m2, "r0": r0}

            def stage_B2(h, I, st):
                m2, r0 = st["m2"], st["r0"]
                mt = pw.tile([128, 1024], F16, tag="wk", name=f"mt_{h}_{I}")
                nc.vector._custom_dve(nsqrt, out=mt[:],
                                      in0=m2[:], in1=r0[:],
                                      s0=1.5, s1=0.5)
                attn = pw.tile([128, 1024], F16, tag="wk",
                               name=f"attn_{h}_{I}")
                rs = psm.tile([128, 1], F32, tag="sm", name=f"rs_{h}_{I}")
                nc.scalar.activation(attn[:], mt[:], AF.Exp, accum_out=rs[:])
                rc = psm.tile([128, 1], F32, tag="sm", name=f"rc_{h}_{I}")
                nc.vector.reciprocal(rc[:], rs[:])
                # normalize on Pool (SBUF-only op), a full step ahead of the
                # transposes that consume attn
                nc.gpsimd.tensor_scalar_mul(attn[:], attn[:], rc[:])
                return {"attn": attn}

            def stage_C(h, I, st):
                attn = st["attn"]
                tps = psC.tile([128, 1024], F16, tag="pc", name=f"tps_{h}_{I}")
                for J in range(NT):
                    js = slice(J * 128, (J + 1) * 128)
                    nc.tensor.transpose(tps[:, js], attn[:, js], id16[:])
                atT = pw.tile([128, 1024], F16, tag="wk", name=f"atT_{h}_{I}")
                nc.vector.tensor_copy(atT[:], tps[:])
                return atT

            def stage_D(h, I, atT):
                isl = slice(I * 128, (I + 1) * 128)
                avs = psC.tile([128, 128], F32, tag="pc", name=f"avs_{h}_{I}")
                vsl = slice((h % 2) * 128, (h % 2) * 128 + 128)
                for J in range(NT):
                    js = slice(J * 128, (J + 1) * 128)
                    nc.tensor.matmul(avs[:], Vpp[h // 2][:, J, vsl],
                                     atT[:, js],
                                     start=(J == 0), stop=(J == NT - 1))
                prt = slice((h % 2) * 64, (h % 2) * 64 + 64)
                nc.vector.tensor_copy(OT[h // 2][prt, isl], avs[0:64, :])
                nc.vector.tensor_copy(OT[2 + h // 2][prt, isl],
                                      avs[64:128, :])

            def emit_outproj(nh):
                ns = slice(nh * 512, (nh + 1) * 512)
                for part, wo_t, bo_t in (("r", wo_sr, bo_rt),
                                         ("i", wo_si, bo_it)):
                    for dt_ in range(4):
                        ds = slice(dt_ * 128, (dt_ + 1) * 128)
                        ops = psC.tile([128, 512], F32, tag="pc",
                                       name=f"ops_{part}_{dt_}_{nh}")
                        for j in range(4):
                            nc.tensor.matmul(ops[:], wo_t[:, j, ds],
                                             OT[j][:, ns],
                                             start=(j == 0), stop=(j == 3))
                        osb = pout.tile([128, 512], F32, tag="ot",
                                        name=f"osb_{part}_{dt_}_{nh}")
                        if nh == 1:
                            # drain tail: DVE empties ~16us before ACT here
                            nc.vector.tensor_scalar_add(osb[:], ops[:],
                                                        bo_t[:, dt_:dt_ + 1])
                        else:
                            nc.scalar.activation(osb[:], ops[:], AF.Identity,
                                                 bias=bo_t[:, dt_:dt_ + 1])
                        dst = o_r if part == "r" else o_i
                        nc.sync.dma_start(
                            bass.AP(dst, dt_ * 128 * N + nh * 512,
                                    [[N, 128], [1, 512]]),
                            osb[:])

            flat = [(h, I) for h in range(HPC) for I in range(NT)]
            PB2, PC, PD = PB + 1, PB + 2, PB + 3
            rmap, skewmap, bmap, b2map, cmap = {}, {}, {}, {}, {}
            # late stages are emitted FIRST within each step so that
            # dependency waits of the stage-B tail never head-of-line block
            # the in-order engine queues for already-runnable work
            for s in range(len(flat) + PD + 1):
                for _ in range(2):
                    if units:
                        u = units.pop(0)
                        if u[0] == "v":
                            emit_vproj(u[1], u[2])
                        else:
                            emit_proj(*u)
                if s < len(flat):
                    h, I = flat[s]
                    rmap[(h, I)] = emit_qrel_r(h, I)
                if 1 <= s < len(flat) + 1:
                    h, I = flat[s - 1]
                    skewmap[(h, I)] = emit_qrel_i(h, I, rmap.pop((h, I)))
                if PB <= s < len(flat) + PB:
                    h, I = flat[s - PB]
                    bmap[(h, I)] = stage_B(h, I, skewmap.pop((h, I)))
                if PB2 <= s < len(flat) + PB2:
                    h, I = flat[s - PB2]
                    b2map[(h, I)] = stage_B2(h, I, bmap.pop((h, I)))
                if PC <= s < len(flat) + PC:
                    h, I = flat[s - PC]
                    cmap[(h, I)] = stage_C(h, I, b2map.pop((h, I)))
                if PD <= s < len(flat) + PD:
                    h, I = flat[s - PD]
                    stage_D(h, I, cmap.pop((h, I)))
                    if (h, I) == (HPC - 1, 3):
                        emit_outproj(0)
            emit_outproj(1)

    nc.compile()
    return nc, addsq


def _prep_core_inputs(inputs, core):
    b, half = core // 2, core % 2
    x = inputs["x"]
    f16 = np.float16
    f32 = np.float32
    xt_r = np.ascontiguousarray(x[b, :, :, 0].T).astype(f16)
    xt_i = np.ascontiguousarray(x[b, :, :, 1].T).astype(f16)

    def pack_ab(wr, wi):
        a = np.empty((DIM, 512), f32)
        bb = np.empty((DIM, 512), f32)
        for hl in range(HPC):
            gh = half * HPC + hl
            cs = slice(gh * DH, (gh + 1) * DH)
            a[:, hl * 128:hl * 128 + 64] = wr[:, cs]
            a[:, hl * 128 + 64:hl * 128 + 128] = wi[:, cs]
            bb[:, hl * 128:hl * 128 + 64] = -wi[:, cs]
            bb[:, hl * 128 + 64:hl * 128 + 128] = wr[:, cs]
        return a.astype(f16), bb.astype(f16)

    wq_a, wq_b = pack_ab(inputs["wq_r"], inputs["wq_i"])
    wk_a, wk_b = pack_ab(inputs["wkv_r"][:, :512], inputs["wkv_i"][:, :512])
    wv_a, wv_b = pack_ab(inputs["wkv_r"][:, 512:], inputs["wkv_i"][:, 512:])

    rs = slice(half * 256, (half + 1) * 256)
    wo_sr = np.concatenate(
        [inputs["wo_r"][rs, :], -inputs["wo_i"][rs, :]], 0).astype(f16)
    wo_si = np.concatenate(
        [inputs["wo_i"][rs, :], inputs["wo_r"][rs, :]], 0).astype(f16)

    e = np.arange(2047)
    t_ext = inputs["rel_emb"][np.clip(e - 1023, -MAX_POS, MAX_POS) + MAX_POS]
    relrev = t_ext[::-1].astype(f32)           # [2047, 64]
    rel_r = np.zeros((128, 2048), f32)
    rel_i = np.zeros((128, 2048), f32)
    rel_r[0:64, 0:2047] = relrev.T
    rel_i[64:128, 0:2047] = -relrev.T

    bscale = 1.0 if half == 0 else 0.0
    bo_rt = np.ascontiguousarray(
        inputs["bo_r"].reshape(4, 128).T * bscale).astype(f32)
    bo_it = np.ascontiguousarray(
        inputs["bo_i"].reshape(4, 128).T * bscale).astype(f32)
    smask = np.concatenate(
        [np.full(64, SCALE, f32), np.full(64, -SCALE, f32)]).reshape(128, 1)

    return {
        "xt_r": xt_r, "xt_i": xt_i,
        "wq_a": wq_a, "wq_b": wq_b, "wk_a": wk_a, "wk_b": wk_b,
        "wv_a": wv_a, "wv_b": wv_b,
        "wo_sr": wo_sr, "wo_si": wo_si,
        "rel_r": rel_r.astype(f16), "rel_i": rel_i.astype(f16),
        "bo_rt": bo_rt, "bo_it": bo_it, "smask": smask,
    }


_last_results = {}


def kernel(**inputs):
    inputs = {k: np.asarray(v) for k, v in inputs.items()}
    nc, _ = build_module()
    in_maps = [_prep_core_inputs(inputs, c) for c in range(8)]
    res = run_bass_kernel_spmd(nc, in_maps, core_ids=list(range(8)))
    _last_results["res"] = res

    out = np.empty((B, N, DIM, 2), np.float32)
    for b in range(B):
        r = res.results[2 * b]["o_r"] + res.results[2 * b + 1]["o_r"]
        i = res.results[2 * b]["o_i"] + res.results[2 * b + 1]["o_i"]
        out[b, :, :, 0] = r.T
        out[b, :, :, 1] = i.T
    return out


# revision 50
# speedup vs baseline: 1.0277x; 1.0274x over previous
"""Complex-valued relative-position attention (nn_CAttention) on 8 TRN2 cores.

Sharding: batch (4) x head-half (2) -> 8 cores. Each core computes its
batch's projections for its 4 heads, full attention for those heads, and a
row-split partial output projection. Host sums the two partial outputs per
batch and restacks.

v2 redesign (engine rebalance, f16 matmul inputs):
  - All matmul moving operands f16 (moving-side dtype sets the PE rate:
    1 cyc/row at any N; baseline's f32r qrel tail chunks ran at 4 cyc/row).
  - Half-inject: the i-part skew lands in the dots_i PSUM group via an
    extra identity-matmul accumulation (PE), so stage-B DVE work is two
    fused custom ops: er=(skw_r+dots_r)^2 (ADDSQ) and m2=er+dots_i^2
    (SQACC), instead of 2x ADDSQ + add.
  - qrel r/i parts pipeline through ONE f32 PSUM slot on alternating
    steps (r at step s, i at step s+1), with the PSUM->SBUF f16 copies
    split: r on ACT, i on DVE, 128-wide tails on Pool.
  - One combined skew DMA write [128,2304] and one 3-level-AP read
    [128,2048] per (h,I), both on the SP HWDGE queue (engine-free).
  - to_out negation folded into host-packed weights ([wo_r; -wo_i] /
    [wo_i; wo_r] row stacks): stage_D needs only 2 PSUM->OT copies (Pool).
  - attn transpose copy (atT) and V copies on Pool; ln/exp/exp softmax
    chain on ACT (pinned ln+exp table set, zero table switches).
"""
import functools
import numpy as np

import concourse.bass as bass
import concourse.bacc as bacc
import concourse.mybir as mybir
import concourse.tile as tile
from concourse.bass_utils import run_bass_kernel_spmd
from concourse.masks import make_identity

F32 = mybir.dt.float32
F16 = mybir.dt.float16
AF = mybir.ActivationFunctionType
ALU = mybir.AluOpType
U16 = mybir.dt.uint16

HEADS, DH, MAX_POS = 8, 64, 512
B, N, DIM = 4, 1024, 512
HPC = 4            # heads per core
KT = 4             # dim k-tiles (512/128)
NT = 8             # n tiles (1024/128)
SCALE = DH ** (-0.5)
PB = 3             # qrel->stage_B pipeline distance (iterations)


def register_custom():
    from concourse import dve_ops
    from concourse.dve_spec import Spec, Src0, Src1, AluOp, Bin, lower, sq
    from concourse.dve_spec import C0, C1
    from concourse.dve_uop import DveOpSpec

    def mk(name, body, ref):
        for op in dve_ops.OPS:
            if op.name == name:
                return op
        spec = Spec(body=body, reference=ref)
        opcode = dve_ops._CUSTOM_DVE_ROW_BASE + len(dve_ops.OPS)
        shas = {}
        for ver in ("v3",):
            s = DveOpSpec(name=name, opcode=opcode,
                          uops=lower(spec, ver=ver), rd1_en=True)
            shas[ver] = s.sha(ver)
        op = dve_ops.DveOp(name, spec, subdim=False, uops_sha=shas)
        dve_ops._SUB_OPCODE_FOR_NAME[op.name] = opcode
        dve_ops.OPS.append(op)
        dve_ops.CUSTOM_DVE_SPECS[op.name] = op.spec
        return op

    addsq = mk(
        "ADDSQ_ANT",
        sq(Bin(AluOp.ADD, Src0, Src1)),
        lambda in0, in1, s0, s1, imm2: (
            (in0.astype(np.float32) + in1.astype(np.float32)) ** 2),
    )
    sqacc = mk(
        "SQACC_ANT",
        Bin(AluOp.ADD, Src0, sq(Src1)),
        lambda in0, in1, s0, s1, imm2: (
            in0.astype(np.float32) + in1.astype(np.float32) ** 2),
    )
    # m = x*r0*(C0 - C1*(x*r0)*r0): one Newton-Raphson rsqrt step folded
    # with the final multiply; Src0 = x (=m2), Src1 = r0 (bit-trick rsqrt
    # seed), C0 = 1.5, C1 = 0.5. Produces sqrt(x) to ~0.2%.
    _t = Bin(AluOp.MULTIPLY, Src0, Src1)
    nsqrt = mk(
        "NSQRT_ANT",
        Bin(AluOp.MULTIPLY, _t,
            Bin(AluOp.SUBTRACT, C0,
                Bin(AluOp.MULTIPLY, C1,
                    Bin(AluOp.MULTIPLY, _t, Src1)))),
        lambda in0, in1, s0, s1, imm2: (
            (in0.astype(np.float32) * in1.astype(np.float32))
            * (s0 - s1 * (in0.astype(np.float32) * in1.astype(np.float32))
               * in1.astype(np.float32))),
    )
    return addsq, sqacc, nsqrt


def c_lo(i_blk):
    return 896 - 128 * i_blk


@functools.cache
def build_module():
    import concourse.tile_utils as tile_utils
    if getattr(tile_utils, "max_sbuf_usage", 0) < 208 * 1024:
        tile_utils.max_sbuf_usage = 208 * 1024

    # Pin the ACT engine to the ln+exp table set: every activation this
    # kernel emits (Ln, Exp, Copy/Identity) lives in that one set, so the
    # greedy table-load pass emits exactly one LoadActFuncSet.
    import concourse.bacc as bacc_mod
    if not getattr(bacc_mod, "_ant_act_tables_pinned", False):
        orig_gat = bacc_mod.get_activation_tables

        def pinned_gat(arch):
            full = orig_gat(arch)
            out = {}
            for name, funcs in full.items():
                if name != "natural_log_exp_and_others":
                    funcs = funcs - {mybir.ActivationFunctionType.Ln,
                                     mybir.ActivationFunctionType.Exp}
                out[name] = funcs
            return out

        bacc_mod.get_activation_tables = pinned_gat
        bacc_mod._ant_act_tables_pinned = True

    addsq, sqacc, nsqrt = register_custom()
    nc = bacc.Bacc("TRN2", target_bir_lowering=False, debug=False,
                   num_devices=8)

    din = {}
    for nm, shape, dt_ in [
        ("xt_r", [DIM, N], F16), ("xt_i", [DIM, N], F16),
        ("wq_a", [DIM, 512], F16), ("wq_b", [DIM, 512], F16),
        ("wk_a", [DIM, 512], F16), ("wk_b", [DIM, 512], F16),
        ("wv_a", [DIM, 512], F16), ("wv_b", [DIM, 512], F16),
        ("wo_sr", [DIM, 512], F16), ("wo_si", [DIM, 512], F16),
        ("rel_r", [128, 2048], F16), ("rel_i", [128, 2048], F16),
        ("bo_rt", [128, 4], F32), ("bo_it", [128, 4], F32),
        ("smask", [128, 1], F32),
    ]:
        din[nm] = nc.dram_tensor(nm, shape, dt_, kind="ExternalInput")
    o_r = nc.dram_tensor("o_r", [DIM, N], F32, kind="ExternalOutput")
    o_i = nc.dram_tensor("o_i", [DIM, N], F32, kind="ExternalOutput")

    with tile.TileContext(nc) as tc:
        with (
            tc.tile_pool(name="const", bufs=1) as cpool,
            tc.tile_pool(name="wts", bufs=6) as pwt,      # 4KB (weights)
            tc.tile_pool(name="xts", bufs=8) as pxt,      # 2KB (x tiles)
            tc.tile_pool(name="work", bufs=30) as pw,     # 2KB slots
            tc.tile_pool(name="qev", bufs=4) as pqe,      # 4.5KB slots
            tc.tile_pool(name="skew", bufs=6) as psk,     # 4KB slots
            tc.tile_pool(name="stacks", bufs=16) as pstk,  # 2KB slots
            tc.tile_pool(name="vstk", bufs=2) as pvp,     # 4KB slots
            tc.tile_pool(name="outsb", bufs=4) as pout,
            tc.tile_pool(name="small", bufs=16) as psm,
            tc.tile_pool(name="psD", bufs=3, space="PSUM") as psD,   # 2-bank
            tc.tile_pool(name="psC", bufs=2, space="PSUM") as psC,   # 1-bank
            tc.tile_pool(name="dram", bufs=7, space="DRAM") as pdram,
        ):
            # ---------------- constants ----------------
            id16 = cpool.tile([128, 128], F16, tag="id16")
            make_identity(nc, id16[:])
            smask = cpool.tile([128, 1], F32, tag="smask")
            nc.sync.dma_start(smask[:], din["smask"][:, :])

            def load_w4(nm):
                # [512, 512] dram -> [128, 4, 512] tile (one DMA)
                t = pwt.tile([128, 4, 512], F16, tag="w4", name=nm)
                nc.sync.dma_start(
                    t[:],
                    bass.AP(din[nm], 0, [[512, 128], [128 * 512, 4], [1, 512]]))
                return t


            # ---------------- phase P: projections ----------------
            A = [None] * HPC
            A2 = [None] * HPC
            Knat = [None] * HPC

            wq = (load_w4("wq_a"), load_w4("wq_b"))

            # xt tiles: [128, 1024] per (r/i, kt)
            xt = {}
            for nm in ("xt_r", "xt_i"):
                for kt in range(KT):
                    t = pxt.tile([128, 1024], F16, tag="xt",
                                 name=f"{nm}_{kt}")
                    nc.sync.dma_start(
                        t[:], bass.AP(din[nm], kt * 128 * N,
                                      [[N, 128], [1, 1024]]))
                    xt[(nm, kt)] = t

            wk = (load_w4("wk_a"), load_w4("wk_b"))
            wva = load_w4("wv_a")
            wvb = load_w4("wv_b")

            rel_r = cpool.tile([128, 2048], F16, tag="rel_r")
            rel_i = cpool.tile([128, 2048], F16, tag="rel_i")
            nc.sync.dma_start(rel_r[:], din["rel_r"][:, :])
            nc.sync.dma_start(rel_i[:], din["rel_i"][:, :])
            bo_rt = cpool.tile([128, 4], F32, tag="bo_rt")
            bo_it = cpool.tile([128, 4], F32, tag="bo_it")
            nc.sync.dma_start(bo_rt[:], din["bo_rt"][:, :])
            nc.sync.dma_start(bo_it[:], din["bo_it"][:, :])
            wo_sr = cpool.tile([128, 4, 512], F16, tag="wo_sr")
            wo_si = cpool.tile([128, 4, 512], F16, tag="wo_si")
            nc.sync.dma_start(
                wo_sr[:],
                bass.AP(din["wo_sr"], 0, [[512, 128], [128 * 512, 4], [1, 512]]))
            nc.sync.dma_start(
                wo_si[:],
                bass.AP(din["wo_si"], 0, [[512, 128], [128 * 512, 4], [1, 512]]))

            def emit_proj(kind, h, nh):
                wa, wb = wq if kind == "q" else wk
                hs = slice(h * 128, (h + 1) * 128)
                if nh == 0:
                    if kind == "q":
                        A[h] = pstk.tile([128, 1024], F16, tag="stk",
                                         name=f"A{h}")
                    else:
                        Knat[h] = pstk.tile([128, 1024], F16, tag="stk",
                                            name=f"Knat{h}")
                ns = slice(nh * 512, (nh + 1) * 512)
                ps = psD.tile([128, 512], F32, tag="pd",
                              name=f"ps{kind}_{h}_{nh}")
                for kt in range(KT):
                    nc.tensor.matmul(ps[:], wa[:, kt, hs],
                                     xt[("xt_r", kt)][:, ns],
                                     start=(kt == 0), stop=False)
                for kt in range(KT):
                    nc.tensor.matmul(ps[:], wb[:, kt, hs],
                                     xt[("xt_i", kt)][:, ns],
                                     start=False, stop=(kt == KT - 1))
                if kind == "q":
                    nc.scalar.mul(A[h][:, ns], ps[:], smask[:])
                else:
                    nc.scalar.copy(Knat[h][:, ns], ps[:])
                if kind == "q" and nh == 1:
                    # A2 = [s*qi; s*qr] built from A = [s*qr; -s*qi] on
                    # Pool (SBUF->SBUF), so dots_i = A2^T @ Knat and the
                    # Kni2 stack is never materialized
                    A2[h] = pstk.tile([128, 1024], F16, tag="stk",
                                      name=f"A2_{h}")
                    nc.gpsimd.tensor_scalar_mul(A2[h][0:64, :],
                                                A[h][64:128, :], -1.0)
                    nc.gpsimd.tensor_copy(A2[h][64:128, :], A[h][0:64, :])

            for kind, h in (("q", 0), ("q", 1), ("k", 0)):
                for nh in range(2):
                    emit_proj(kind, h, nh)
            Vpp = [pvp.tile([128, 8, 256], F16, tag="vs", name=f"Vpp{p}")
                   for p in range(2)]

            def emit_vproj(p, J):
                # two heads per matmul (256-wide output)
                hs = slice(p * 256, (p + 1) * 256)
                js = slice(J * 128, (J + 1) * 128)
                vps = psC.tile([128, 256], F32, tag="pc",
                               name=f"vps_{p}_{J}")
                for kt in range(KT):
                    nc.tensor.matmul(vps[:],
                                     xt[("xt_r", kt)][:, js],
                                     wva[:, kt, hs],
                                     start=(kt == 0), stop=False)
                for kt in range(KT):
                    nc.tensor.matmul(vps[:],
                                     xt[("xt_i", kt)][:, js],
                                     wvb[:, kt, hs],
                                     start=False, stop=(kt == KT - 1))
                nc.scalar.copy(Vpp[p][:, J, :], vps[:])

            units = ([("k", 1, 0), ("k", 1, 1)]
                     + [("v", 0, J) for J in range(NT)]
                     + [(k, h, nh) for h in (2, 3) for k in ("q", "k")
                        for nh in range(2)]
                     + [("v", 1, J) for J in range(NT)])

            # OT stacks for the output projection:
            # OT[0] = or heads 0,1 | OT[1] = or heads 2,3
            # OT[2] = oi heads 0,1 | OT[3] = oi heads 2,3
            OT = [pstk.tile([128, 1024], F16, tag="stk", name=f"OT{t}")
                  for t in range(4)]

            # ---------------- phase A: pipelined attention ----------------
            # Per (h, I): qrel r-part at step s, i-part + skew write/read at
            # step s+1, dots + softmax at s+PB, transpose at s+PB+2, AV at
            # s+PB+3.
            def emit_qrel_r(h, I):
                isl = slice(I * 128, (I + 1) * 128)
                lo = c_lo(I)
                qe = pqe.tile([128, 2, 1152], F16, tag="qe",
                              name=f"qe_{h}_{I}")
                qtl = psC.tile([128, 2, 128], F32, tag="pc",
                               name=f"qtl_{h}_{I}")
                qps = psD.tile([128, 1024], F32, tag="pd",
                               name=f"qpsr_{h}_{I}")
                for c0 in (0, 512):
                    nc.tensor.matmul(qps[:, c0:c0 + 512], A[h][:, isl],
                                     rel_r[:, lo + c0:lo + c0 + 512],
                                     start=True, stop=True)
                nc.tensor.matmul(qtl[:, 0, :], A[h][:, isl],
                                 rel_r[:, lo + 1024:lo + 1152],
                                 start=True, stop=True)
                nc.scalar.copy(qe[:, 0, 0:1024], qps[:])
                return {"qe": qe, "qtl": qtl}

            def emit_qrel_i(h, I, st):
                isl = slice(I * 128, (I + 1) * 128)
                lo = c_lo(I)
                qe, qtl = st["qe"], st["qtl"]
                qps = psD.tile([128, 1024], F32, tag="pd",
                               name=f"qpsi_{h}_{I}")
                for c0 in (0, 512):
                    nc.tensor.matmul(qps[:, c0:c0 + 512], A[h][:, isl],
                                     rel_i[:, lo + c0:lo + c0 + 512],
                                     start=True, stop=True)
                nc.tensor.matmul(qtl[:, 1, :], A[h][:, isl],
                                 rel_i[:, lo + 1024:lo + 1152],
                                 start=True, stop=True)
                nc.scalar.copy(qe[:, 1, 0:1024], qps[:])
                nc.scalar.copy(qe[:, :, 1024:1152], qtl[:])
                slot = pdram.tile([128, 2304], F16, tag="qrev",
                                  name=f"qrev_{h}_{I}")
                nc.sync.dma_start(slot[:, :], qe[:])
                skw = psk.tile([128, 2048], F16, tag="skw",
                               name=f"skew_{h}_{I}")
                nc.sync.dma_start(
                    skw[:],
                    bass.AP(slot.tensor, 127,
                            [[2303, 128], [1152, 2], [1, 1024]]))
                return skw

            def stage_B(h, I, skw):
                isl = slice(I * 128, (I + 1) * 128)
                dpsr = psD.tile([128, 1024], F32, tag="pd",
                                name=f"dpsr_{h}_{I}")
                dpsi = psD.tile([128, 1024], F32, tag="pd",
                                name=f"dpsi_{h}_{I}")
                # i-part skew injected into the dots_i PSUM group via
                # identity-matmul accumulation (DVE can read only one PSUM
                # operand per op, so r-part adds via the fused ADDSQ)
                for nh in range(2):
                    ns = slice(nh * 512, (nh + 1) * 512)
                    nc.tensor.matmul(dpsr[:, ns], A[h][:, isl],
                                     Knat[h][:, ns], start=True, stop=True)
                    nc.tensor.matmul(dpsi[:, ns], A2[h][:, isl],
                                     Knat[h][:, ns], start=True, stop=False)
                    nc.tensor.matmul(dpsi[:, ns], id16[:],
                                     skw[:, 1024 + nh * 512:1024 + nh * 512 + 512],
                                     start=False, stop=True)
                er = pw.tile([128, 1024], F16, tag="wk", name=f"er_{h}_{I}")
                nc.vector._custom_dve(addsq, out=er[:],
                                      in0=skw[:, 0:1024], in1=dpsr[:])
                m2 = pw.tile([128, 1024], F16, tag="wk", name=f"m2_{h}_{I}")
                nc.vector._custom_dve(sqacc, out=m2[:],
                                      in0=er[:], in1=dpsi[:])
                # sqrt(m2) without the ACT ln/exp round trip: integer-view
                # rsqrt bit-seed on Pool, then one fused Newton step * m2 on
                # DVE (NSQRT, next pipeline step).  bits(r0) = 22971 -
                # bits(m2)/2.
                r0 = pw.tile([128, 1024], F16, tag="wk", name=f"r0_{h}_{I}")
                nc.gpsimd.tensor_scalar(
                    r0[:].bitcast(U16), m2[:].bitcast(U16),
                    -0.5, 22971.0, ALU.mult, ALU.add)
                return {"m2": m2, "r0": r0}

            def stage_B2(h, I, st):
                m2, r0 = st["m2"], st["r0"]
                mt = pw.tile([128, 1024], F16, tag="wk", name=f"mt_{h}_{I}")
                nc.vector._custom_dve(nsqrt, out=mt[:],
                                      in0=m2[:], in1=r0[:],
                                      s0=1.5, s1=0.5)
                attn = pw.tile([128, 1024], F16, tag="wk",
                               name=f"attn_{h}_{I}")
                rs = psm.tile([128, 1], F32, tag="sm", name=f"rs_{h}_{I}")
                nc.scalar.activation(attn[:], mt[:], AF.Exp, accum_out=rs[:])
                rc = psm.tile([128, 1], F32, tag="sm", name=f"rc_{h}_{I}")
                nc.vector.reciprocal(rc[:], rs[:])
                # normalize on Pool (SBUF-only op), a full step ahead of the
                # transposes that consume attn
                nc.gpsimd.tensor_scalar_mul(attn[:], attn[:], rc[:])
                return {"attn": attn}

            def stage_C(h, I, st):
                attn = st["attn"]
                tps = psC.tile([128, 1024], F16, tag="pc", name=f"tps_{h}_{I}")
                for J in range(NT):
                    js = slice(J * 128, (J + 1) * 128)
                    nc.tensor.transpose(tps[:, js], attn[:, js], id16[:])
                atT = pw.tile([128, 1024], F16, tag="wk", name=f"atT_{h}_{I}")
                nc.vector.tensor_copy(atT[:], tps[:])
                return atT

            def stage_D(h, I, atT):
                isl = slice(I * 128, (I + 1) * 128)
                avs = psC.tile([128, 128], F32, tag="pc", name=f"avs_{h}_{I}")
                vsl = slice((h % 2) * 128, (h % 2) * 128 + 128)
                for J in range(NT):
                    js = slice(J * 128, (J + 1) * 128)
                    nc.tensor.matmul(avs[:], Vpp[h // 2][:, J, vsl],
                                     atT[:, js],
                                     start=(J == 0), stop=(J == NT - 1))
                prt = slice((h % 2) * 64, (h % 2) * 64 + 64)
                nc.vector.tensor_copy(OT[h // 2][prt, isl], avs[0:64, :])
                nc.vector.tensor_copy(OT[2 + h // 2][prt, isl],
                                      avs[64:128, :])

            def emit_outproj_unit(nh, part, dt_):
                ns = slice(nh * 512, (nh + 1) * 512)
                wo_t, bo_t = (wo_sr, bo_rt) if part == "r" else (wo_si, bo_it)
                if True:
                    if True:
                        ds = slice(dt_ * 128, (dt_ + 1) * 128)
                        ops = psC.tile([128, 512], F32, tag="pc",
                                       name=f"ops_{part}_{dt_}_{nh}")
                        for j in range(4):
                            nc.tensor.matmul(ops[:], wo_t[:, j, ds],
                                             OT[j][:, ns],
                                             start=(j == 0), stop=(j == 3))
                        osb = pout.tile([128, 512], F32, tag="ot",
                                        name=f"osb_{part}_{dt_}_{nh}")
                        if nh == 1:
                            # drain tail: DVE empties ~16us before ACT here
                            nc.vector.tensor_scalar_add(osb[:], ops[:],
                                                        bo_t[:, dt_:dt_ + 1])
                        else:
                            nc.scalar.activation(osb[:], ops[:], AF.Identity,
                                                 bias=bo_t[:, dt_:dt_ + 1])
                        dst = o_r if part == "r" else o_i
                        nc.sync.dma_start(
                            bass.AP(dst, dt_ * 128 * N + nh * 512,
                                    [[N, 128], [1, 512]]),
                            osb[:])

            def emit_outproj(nh):
                for part in ("r", "i"):
                    for dt_ in range(4):
                        emit_outproj_unit(nh, part, dt_)

            ounits = []

            flat = [(h, I) for h in range(HPC) for I in range(NT)]
            PB2, PC, PD = PB + 1, PB + 2, PB + 3
            rmap, skewmap, bmap, b2map, cmap = {}, {}, {}, {}, {}
            # late stages are emitted FIRST within each step so that
            # dependency waits of the stage-B tail never head-of-line block
            # the in-order engine queues for already-runnable work
            for s in range(len(flat) + PD + 1):
                for _ in range(2):
                    if units:
                        u = units.pop(0)
                        if u[0] == "v":
                            emit_vproj(u[1], u[2])
                        else:
                            emit_proj(*u)
                if s < len(flat):
                    h, I = flat[s]
                    rmap[(h, I)] = emit_qrel_r(h, I)
                if 1 <= s < len(flat) + 1:
                    h, I = flat[s - 1]
                    skewmap[(h, I)] = emit_qrel_i(h, I, rmap.pop((h, I)))
                if PB <= s < len(flat) + PB:
                    h, I = flat[s - PB]
                    bmap[(h, I)] = stage_B(h, I, skewmap.pop((h, I)))
                if PB2 <= s < len(flat) + PB2:
                    h, I = flat[s - PB2]
                    b2map[(h, I)] = stage_B2(h, I, bmap.pop((h, I)))
                if PC <= s < len(flat) + PC:
                    h, I = flat[s - PC]
                    cmap[(h, I)] = stage_C(h, I, b2map.pop((h, I)))
                for _ in range(2):
                    if ounits:
                        emit_outproj_unit(*ounits.pop(0))
                if PD <= s < len(flat) + PD:
                    h, I = flat[s - PD]
                    stage_D(h, I, cmap.pop((h, I)))
                    if (h, I) == (HPC - 1, 3):
                        ounits.extend((0, part, dt_) for part in ("r", "i")
                                      for dt_ in range(4))
            while ounits:
                emit_outproj_unit(*ounits.pop(0))
            emit_outproj(1)

    nc.compile()
    return nc, addsq


def _prep_core_inputs(inputs, core):
    b, half = core // 2, core % 2
    x = inputs["x"]
    f16 = np.float16
    f32 = np.float32
    xt_r = np.ascontiguousarray(x[b, :, :, 0].T).astype(f16)
    xt_i = np.ascontiguousarray(x[b, :, :, 1].T).astype(f16)

    def pack_ab(wr, wi):
        a = np.empty((DIM, 512), f32)
        bb = np.empty((DIM, 512), f32)
        for hl in range(HPC):
            gh = half * HPC + hl
            cs = slice(gh * DH, (gh + 1) * DH)
            a[:, hl * 128:hl * 128 + 64] = wr[:, cs]
            a[:, hl * 128 + 64:hl * 128 + 128] = wi[:, cs]
            bb[:, hl * 128:hl * 128 + 64] = -wi[:, cs]
            bb[:, hl * 128 + 64:hl * 128 + 128] = wr[:, cs]
        return a.astype(f16), bb.astype(f16)

    wq_a, wq_b = pack_ab(inputs["wq_r"], inputs["wq_i"])
    wk_a, wk_b = pack_ab(inputs["wkv_r"][:, :512], inputs["wkv_i"][:, :512])
    wv_a, wv_b = pack_ab(inputs["wkv_r"][:, 512:], inputs["wkv_i"][:, 512:])

    rs = slice(half * 256, (half + 1) * 256)
    wo_sr = np.concatenate(
        [inputs["wo_r"][rs, :], -inputs["wo_i"][rs, :]], 0).astype(f16)
    wo_si = np.concatenate(
        [inputs["wo_i"][rs, :], inputs["wo_r"][rs, :]], 0).astype(f16)

    e = np.arange(2047)
    t_ext = inputs["rel_emb"][np.clip(e - 1023, -MAX_POS, MAX_POS) + MAX_POS]
    relrev = t_ext[::-1].astype(f32)           # [2047, 64]
    rel_r = np.zeros((128, 2048), f32)
    rel_i = np.zeros((128, 2048), f32)
    rel_r[0:64, 0:2047] = relrev.T
    rel_i[64:128, 0:2047] = -relrev.T

    bscale = 1.0 if half == 0 else 0.0
    bo_rt = np.ascontiguousarray(
        inputs["bo_r"].reshape(4, 128).T * bscale).astype(f32)
    bo_it = np.ascontiguousarray(
        inputs["bo_i"].reshape(4, 128).T * bscale).astype(f32)
    smask = np.concatenate(
        [np.full(64, SCALE, f32), np.full(64, -SCALE, f32)]).reshape(128, 1)

    return {
        "xt_r": xt_r, "xt_i": xt_i,
        "wq_a": wq_a, "wq_b": wq_b, "wk_a": wk_a, "wk_b": wk_b,
        "wv_a": wv_a, "wv_b": wv_b,
        "wo_sr": wo_sr, "wo_si": wo_si,
        "rel_r": rel_r.astype(f16), "rel_i": rel_i.astype(f16),
        "bo_rt": bo_rt, "bo_it": bo_it, "smask": smask,
    }


_last_results = {}


def kernel(**inputs):
    inputs = {k: np.asarray(v) for k, v in inputs.items()}
    nc, _ = build_module()
    in_maps = [_prep_core_inputs(inputs, c) for c in range(8)]
    res = run_bass_kernel_spmd(nc, in_maps, core_ids=list(range(8)))
    _last_results["res"] = res

    out = np.empty((B, N, DIM, 2), np.float32)
    for b in range(B):
        r = res.results[2 * b]["o_r"] + res.results[2 * b + 1]["o_r"]
        i = res.results[2 * b]["o_i"] + res.results[2 * b + 1]["o_i"]
        out[b, :, :, 0] = r.T
        out[b, :, :, 1] = i.T
    return out


# revision 51
# speedup vs baseline: 1.0292x; 1.0015x over previous
"""Complex-valued relative-position attention (nn_CAttention) on 8 TRN2 cores.

Sharding: batch (4) x head-half (2) -> 8 cores. Each core computes its
batch's projections for its 4 heads, full attention for those heads, and a
row-split partial output projection. Host sums the two partial outputs per
batch and restacks.

v2 redesign (engine rebalance, f16 matmul inputs):
  - All matmul moving operands f16 (moving-side dtype sets the PE rate:
    1 cyc/row at any N; baseline's f32r qrel tail chunks ran at 4 cyc/row).
  - Half-inject: the i-part skew lands in the dots_i PSUM group via an
    extra identity-matmul accumulation (PE), so stage-B DVE work is two
    fused custom ops: er=(skw_r+dots_r)^2 (ADDSQ) and m2=er+dots_i^2
    (SQACC), instead of 2x ADDSQ + add.
  - qrel r/i parts pipeline through ONE f32 PSUM slot on alternating
    steps (r at step s, i at step s+1), with the PSUM->SBUF f16 copies
    split: r on ACT, i on DVE, 128-wide tails on Pool.
  - One combined skew DMA write [128,2304] and one 3-level-AP read
    [128,2048] per (h,I), both on the SP HWDGE queue (engine-free).
  - to_out negation folded into host-packed weights ([wo_r; -wo_i] /
    [wo_i; wo_r] row stacks): stage_D needs only 2 PSUM->OT copies (Pool).
  - attn transpose copy (atT) and V copies on Pool; ln/exp/exp softmax
    chain on ACT (pinned ln+exp table set, zero table switches).
"""
import functools
import numpy as np

import concourse.bass as bass
import concourse.bacc as bacc
import concourse.mybir as mybir
import concourse.tile as tile
from concourse.bass_utils import run_bass_kernel_spmd
from concourse.masks import make_identity

F32 = mybir.dt.float32
F16 = mybir.dt.float16
AF = mybir.ActivationFunctionType
ALU = mybir.AluOpType
U16 = mybir.dt.uint16

HEADS, DH, MAX_POS = 8, 64, 512
B, N, DIM = 4, 1024, 512
HPC = 4            # heads per core
KT = 4             # dim k-tiles (512/128)
NT = 8             # n tiles (1024/128)
SCALE = DH ** (-0.5)
PB = 3             # qrel->stage_B pipeline distance (iterations)


def register_custom():
    from concourse import dve_ops
    from concourse.dve_spec import Spec, Src0, Src1, AluOp, Bin, lower, sq
    from concourse.dve_spec import C0, C1
    from concourse.dve_uop import DveOpSpec

    def mk(name, body, ref):
        for op in dve_ops.OPS:
            if op.name == name:
                return op
        spec = Spec(body=body, reference=ref)
        opcode = dve_ops._CUSTOM_DVE_ROW_BASE + len(dve_ops.OPS)
        shas = {}
        for ver in ("v3",):
            s = DveOpSpec(name=name, opcode=opcode,
                          uops=lower(spec, ver=ver), rd1_en=True)
            shas[ver] = s.sha(ver)
        op = dve_ops.DveOp(name, spec, subdim=False, uops_sha=shas)
        dve_ops._SUB_OPCODE_FOR_NAME[op.name] = opcode
        dve_ops.OPS.append(op)
        dve_ops.CUSTOM_DVE_SPECS[op.name] = op.spec
        return op

    addsq = mk(
        "ADDSQ_ANT",
        sq(Bin(AluOp.ADD, Src0, Src1)),
        lambda in0, in1, s0, s1, imm2: (
            (in0.astype(np.float32) + in1.astype(np.float32)) ** 2),
    )
    sqacc = mk(
        "SQACC_ANT",
        Bin(AluOp.ADD, Src0, sq(Src1)),
        lambda in0, in1, s0, s1, imm2: (
            in0.astype(np.float32) + in1.astype(np.float32) ** 2),
    )
    # m = x*r0*(C0 - C1*(x*r0)*r0): one Newton-Raphson rsqrt step folded
    # with the final multiply; Src0 = x (=m2), Src1 = r0 (bit-trick rsqrt
    # seed), C0 = 1.5, C1 = 0.5. Produces sqrt(x) to ~0.2%.
    _t = Bin(AluOp.MULTIPLY, Src0, Src1)
    nsqrt = mk(
        "NSQRT_ANT",
        Bin(AluOp.MULTIPLY, _t,
            Bin(AluOp.SUBTRACT, C0,
                Bin(AluOp.MULTIPLY, C1,
                    Bin(AluOp.MULTIPLY, _t, Src1)))),
        lambda in0, in1, s0, s1, imm2: (
            (in0.astype(np.float32) * in1.astype(np.float32))
            * (s0 - s1 * (in0.astype(np.float32) * in1.astype(np.float32))
               * in1.astype(np.float32))),
    )
    return addsq, sqacc, nsqrt


def c_lo(i_blk):
    return 896 - 128 * i_blk


@functools.cache
def build_module():
    import concourse.tile_utils as tile_utils
    if getattr(tile_utils, "max_sbuf_usage", 0) < 208 * 1024:
        tile_utils.max_sbuf_usage = 208 * 1024

    # Pin the ACT engine to the ln+exp table set: every activation this
    # kernel emits (Ln, Exp, Copy/Identity) lives in that one set, so the
    # greedy table-load pass emits exactly one LoadActFuncSet.
    import concourse.bacc as bacc_mod
    if not getattr(bacc_mod, "_ant_act_tables_pinned", False):
        orig_gat = bacc_mod.get_activation_tables

        def pinned_gat(arch):
            full = orig_gat(arch)
            out = {}
            for name, funcs in full.items():
                if name != "natural_log_exp_and_others":
                    funcs = funcs - {mybir.ActivationFunctionType.Ln,
                                     mybir.ActivationFunctionType.Exp}
                out[name] = funcs
            return out

        bacc_mod.get_activation_tables = pinned_gat
        bacc_mod._ant_act_tables_pinned = True

    addsq, sqacc, nsqrt = register_custom()
    nc = bacc.Bacc("TRN2", target_bir_lowering=False, debug=False,
                   num_devices=8)

    din = {}
    for nm, shape, dt_ in [
        ("xt_r", [DIM, N], F16), ("xt_i", [DIM, N], F16),
        ("wq_a", [DIM, 512], F16), ("wq_b", [DIM, 512], F16),
        ("wk_a", [DIM, 512], F16), ("wk_b", [DIM, 512], F16),
        ("wv_a", [DIM, 512], F16), ("wv_b", [DIM, 512], F16),
        ("wo_sr", [DIM, 512], F16), ("wo_si", [DIM, 512], F16),
        ("rel_r", [128, 2048], F16), ("rel_i", [128, 2048], F16),
        ("bo_rt", [128, 4], F32), ("bo_it", [128, 4], F32),
        ("smask", [128, 1], F32),
    ]:
        din[nm] = nc.dram_tensor(nm, shape, dt_, kind="ExternalInput")
    o_r = nc.dram_tensor("o_r", [DIM, N], F32, kind="ExternalOutput")
    o_i = nc.dram_tensor("o_i", [DIM, N], F32, kind="ExternalOutput")

    with tile.TileContext(nc) as tc:
        with (
            tc.tile_pool(name="const", bufs=1) as cpool,
            tc.tile_pool(name="wts", bufs=6) as pwt,      # 4KB (weights)
            tc.tile_pool(name="xts", bufs=8) as pxt,      # 2KB (x tiles)
            tc.tile_pool(name="work", bufs=28) as pw,     # 2KB slots
            tc.tile_pool(name="qev", bufs=4) as pqe,      # 4.5KB slots
            tc.tile_pool(name="skew", bufs=6) as psk,     # 4KB slots
            tc.tile_pool(name="stacks", bufs=16) as pstk,  # 2KB slots
            tc.tile_pool(name="vstk", bufs=2) as pvp,     # 4KB slots
            tc.tile_pool(name="outsb", bufs=6) as pout,
            tc.tile_pool(name="small", bufs=16) as psm,
            tc.tile_pool(name="psD", bufs=3, space="PSUM") as psD,   # 2-bank
            tc.tile_pool(name="psC", bufs=2, space="PSUM") as psC,   # 1-bank
            tc.tile_pool(name="dram", bufs=7, space="DRAM") as pdram,
        ):
            # ---------------- constants ----------------
            id16 = cpool.tile([128, 128], F16, tag="id16")
            make_identity(nc, id16[:])
            smask = cpool.tile([128, 1], F32, tag="smask")
            nc.sync.dma_start(smask[:], din["smask"][:, :])

            def load_w4(nm):
                # [512, 512] dram -> [128, 4, 512] tile (one DMA)
                t = pwt.tile([128, 4, 512], F16, tag="w4", name=nm)
                nc.sync.dma_start(
                    t[:],
                    bass.AP(din[nm], 0, [[512, 128], [128 * 512, 4], [1, 512]]))
                return t


            # ---------------- phase P: projections ----------------
            A = [None] * HPC
            A2 = [None] * HPC
            Knat = [None] * HPC

            wq = (load_w4("wq_a"), load_w4("wq_b"))

            # xt tiles: [128, 1024] per (r/i, kt)
            xt = {}
            for nm in ("xt_r", "xt_i"):
                for kt in range(KT):
                    t = pxt.tile([128, 1024], F16, tag="xt",
                                 name=f"{nm}_{kt}")
                    nc.sync.dma_start(
                        t[:], bass.AP(din[nm], kt * 128 * N,
                                      [[N, 128], [1, 1024]]))
                    xt[(nm, kt)] = t

            wk = (load_w4("wk_a"), load_w4("wk_b"))
            wva = load_w4("wv_a")
            wvb = load_w4("wv_b")

            rel_r = cpool.tile([128, 2048], F16, tag="rel_r")
            rel_i = cpool.tile([128, 2048], F16, tag="rel_i")
            nc.sync.dma_start(rel_r[:], din["rel_r"][:, :])
            nc.sync.dma_start(rel_i[:], din["rel_i"][:, :])
            bo_rt = cpool.tile([128, 4], F32, tag="bo_rt")
            bo_it = cpool.tile([128, 4], F32, tag="bo_it")
            nc.sync.dma_start(bo_rt[:], din["bo_rt"][:, :])
            nc.sync.dma_start(bo_it[:], din["bo_it"][:, :])
            wo_sr = cpool.tile([128, 4, 512], F16, tag="wo_sr")
            wo_si = cpool.tile([128, 4, 512], F16, tag="wo_si")
            nc.sync.dma_start(
                wo_sr[:],
                bass.AP(din["wo_sr"], 0, [[512, 128], [128 * 512, 4], [1, 512]]))
            nc.sync.dma_start(
                wo_si[:],
                bass.AP(din["wo_si"], 0, [[512, 128], [128 * 512, 4], [1, 512]]))

            def emit_proj(kind, h, nh):
                wa, wb = wq if kind == "q" else wk
                hs = slice(h * 128, (h + 1) * 128)
                if nh == 0:
                    if kind == "q":
                        A[h] = pstk.tile([128, 1024], F16, tag="stk",
                                         name=f"A{h}")
                    else:
                        Knat[h] = pstk.tile([128, 1024], F16, tag="stk",
                                            name=f"Knat{h}")
                ns = slice(nh * 512, (nh + 1) * 512)
                ps = psD.tile([128, 512], F32, tag="pd",
                              name=f"ps{kind}_{h}_{nh}")
                for kt in range(KT):
                    nc.tensor.matmul(ps[:], wa[:, kt, hs],
                                     xt[("xt_r", kt)][:, ns],
                                     start=(kt == 0), stop=False)
                for kt in range(KT):
                    nc.tensor.matmul(ps[:], wb[:, kt, hs],
                                     xt[("xt_i", kt)][:, ns],
                                     start=False, stop=(kt == KT - 1))
                if kind == "q":
                    nc.scalar.mul(A[h][:, ns], ps[:], smask[:])
                else:
                    nc.scalar.copy(Knat[h][:, ns], ps[:])
                if kind == "q" and nh == 1:
                    # A2 = [s*qi; s*qr] built from A = [s*qr; -s*qi] on
                    # Pool (SBUF->SBUF), so dots_i = A2^T @ Knat and the
                    # Kni2 stack is never materialized
                    A2[h] = pstk.tile([128, 1024], F16, tag="stk",
                                      name=f"A2_{h}")
                    nc.gpsimd.tensor_scalar_mul(A2[h][0:64, :],
                                                A[h][64:128, :], -1.0)
                    nc.gpsimd.tensor_copy(A2[h][64:128, :], A[h][0:64, :])

            for kind, h in (("q", 0), ("q", 1), ("k", 0)):
                for nh in range(2):
                    emit_proj(kind, h, nh)
            Vpp = [pvp.tile([128, 8, 256], F16, tag="vs", name=f"Vpp{p}")
                   for p in range(2)]

            def emit_vproj(p, J):
                # two heads per matmul (256-wide output)
                hs = slice(p * 256, (p + 1) * 256)
                js = slice(J * 128, (J + 1) * 128)
                vps = psC.tile([128, 256], F32, tag="pc",
                               name=f"vps_{p}_{J}")
                for kt in range(KT):
                    nc.tensor.matmul(vps[:],
                                     xt[("xt_r", kt)][:, js],
                                     wva[:, kt, hs],
                                     start=(kt == 0), stop=False)
                for kt in range(KT):
                    nc.tensor.matmul(vps[:],
                                     xt[("xt_i", kt)][:, js],
                                     wvb[:, kt, hs],
                                     start=False, stop=(kt == KT - 1))
                nc.scalar.copy(Vpp[p][:, J, :], vps[:])

            units = ([("k", 1, 0), ("k", 1, 1)]
                     + [("v", 0, J) for J in range(NT)]
                     + [(k, h, nh) for h in (2, 3) for k in ("q", "k")
                        for nh in range(2)]
                     + [("v", 1, J) for J in range(NT)])

            # OT stacks for the output projection:
            # OT[0] = or heads 0,1 | OT[1] = or heads 2,3
            # OT[2] = oi heads 0,1 | OT[3] = oi heads 2,3
            OT = [pstk.tile([128, 1024], F16, tag="stk", name=f"OT{t}")
                  for t in range(4)]

            # ---------------- phase A: pipelined attention ----------------
            # Per (h, I): qrel r-part at step s, i-part + skew write/read at
            # step s+1, dots + softmax at s+PB, transpose at s+PB+2, AV at
            # s+PB+3.
            def emit_qrel_r(h, I):
                isl = slice(I * 128, (I + 1) * 128)
                lo = c_lo(I)
                qe = pqe.tile([128, 2, 1152], F16, tag="qe",
                              name=f"qe_{h}_{I}")
                qtl = psC.tile([128, 2, 128], F32, tag="pc",
                               name=f"qtl_{h}_{I}")
                qps = psD.tile([128, 1024], F32, tag="pd",
                               name=f"qpsr_{h}_{I}")
                for c0 in (0, 512):
                    nc.tensor.matmul(qps[:, c0:c0 + 512], A[h][:, isl],
                                     rel_r[:, lo + c0:lo + c0 + 512],
                                     start=True, stop=True)
                nc.tensor.matmul(qtl[:, 0, :], A[h][:, isl],
                                 rel_r[:, lo + 1024:lo + 1152],
                                 start=True, stop=True)
                nc.scalar.copy(qe[:, 0, 0:1024], qps[:])
                return {"qe": qe, "qtl": qtl}

            def emit_qrel_i(h, I, st):
                isl = slice(I * 128, (I + 1) * 128)
                lo = c_lo(I)
                qe, qtl = st["qe"], st["qtl"]
                qps = psD.tile([128, 1024], F32, tag="pd",
                               name=f"qpsi_{h}_{I}")
                for c0 in (0, 512):
                    nc.tensor.matmul(qps[:, c0:c0 + 512], A[h][:, isl],
                                     rel_i[:, lo + c0:lo + c0 + 512],
                                     start=True, stop=True)
                nc.tensor.matmul(qtl[:, 1, :], A[h][:, isl],
                                 rel_i[:, lo + 1024:lo + 1152],
                                 start=True, stop=True)
                nc.scalar.copy(qe[:, 1, 0:1024], qps[:])
                nc.scalar.copy(qe[:, :, 1024:1152], qtl[:])
                slot = pdram.tile([128, 2304], F16, tag="qrev",
                                  name=f"qrev_{h}_{I}")
                nc.sync.dma_start(slot[:, :], qe[:])
                skw = psk.tile([128, 2048], F16, tag="skw",
                               name=f"skew_{h}_{I}")
                nc.sync.dma_start(
                    skw[:],
                    bass.AP(slot.tensor, 127,
                            [[2303, 128], [1152, 2], [1, 1024]]))
                return skw

            def stage_B(h, I, skw):
                isl = slice(I * 128, (I + 1) * 128)
                dpsr = psD.tile([128, 1024], F32, tag="pd",
                                name=f"dpsr_{h}_{I}")
                dpsi = psD.tile([128, 1024], F32, tag="pd",
                                name=f"dpsi_{h}_{I}")
                # i-part skew injected into the dots_i PSUM group via
                # identity-matmul accumulation (DVE can read only one PSUM
                # operand per op, so r-part adds via the fused ADDSQ)
                for nh in range(2):
                    ns = slice(nh * 512, (nh + 1) * 512)
                    nc.tensor.matmul(dpsr[:, ns], A[h][:, isl],
                                     Knat[h][:, ns], start=True, stop=True)
                    nc.tensor.matmul(dpsi[:, ns], A2[h][:, isl],
                                     Knat[h][:, ns], start=True, stop=False)
                    nc.tensor.matmul(dpsi[:, ns], id16[:],
                                     skw[:, 1024 + nh * 512:1024 + nh * 512 + 512],
                                     start=False, stop=True)
                er = pw.tile([128, 1024], F16, tag="wk", name=f"er_{h}_{I}")
                nc.vector._custom_dve(addsq, out=er[:],
                                      in0=skw[:, 0:1024], in1=dpsr[:])
                m2 = pw.tile([128, 1024], F16, tag="wk", name=f"m2_{h}_{I}")
                nc.vector._custom_dve(sqacc, out=m2[:],
                                      in0=er[:], in1=dpsi[:])
                # sqrt(m2) without the ACT ln/exp round trip: integer-view
                # rsqrt bit-seed on Pool, then one fused Newton step * m2 on
                # DVE (NSQRT, next pipeline step).  bits(r0) = 22971 -
                # bits(m2)/2.
                r0 = pw.tile([128, 1024], F16, tag="wk", name=f"r0_{h}_{I}")
                nc.gpsimd.tensor_scalar(
                    r0[:].bitcast(U16), m2[:].bitcast(U16),
                    -0.5, 22971.0, ALU.mult, ALU.add)
                return {"m2": m2, "r0": r0}

            def stage_B2(h, I, st):
                m2, r0 = st["m2"], st["r0"]
                mt = pw.tile([128, 1024], F16, tag="wk", name=f"mt_{h}_{I}")
                nc.vector._custom_dve(nsqrt, out=mt[:],
                                      in0=m2[:], in1=r0[:],
                                      s0=1.5, s1=0.5)
                attn = pw.tile([128, 1024], F16, tag="wk",
                               name=f"attn_{h}_{I}")
                rs = psm.tile([128, 1], F32, tag="sm", name=f"rs_{h}_{I}")
                nc.scalar.activation(attn[:], mt[:], AF.Exp, accum_out=rs[:])
                rc = psm.tile([128, 1], F32, tag="sm", name=f"rc_{h}_{I}")
                nc.vector.reciprocal(rc[:], rs[:])
                # normalize on Pool (SBUF-only op), a full step ahead of the
                # transposes that consume attn
                nc.gpsimd.tensor_scalar_mul(attn[:], attn[:], rc[:])
                return {"attn": attn}

            def stage_C(h, I, st):
                attn = st["attn"]
                tps = psC.tile([128, 1024], F16, tag="pc", name=f"tps_{h}_{I}")
                for J in range(NT):
                    js = slice(J * 128, (J + 1) * 128)
                    nc.tensor.transpose(tps[:, js], attn[:, js], id16[:])
                atT = pw.tile([128, 1024], F16, tag="wk", name=f"atT_{h}_{I}")
                nc.vector.tensor_copy(atT[:], tps[:])
                return atT

            def stage_D(h, I, atT):
                isl = slice(I * 128, (I + 1) * 128)
                avs = psC.tile([128, 128], F32, tag="pc", name=f"avs_{h}_{I}")
                vsl = slice((h % 2) * 128, (h % 2) * 128 + 128)
                for J in range(NT):
                    js = slice(J * 128, (J + 1) * 128)
                    nc.tensor.matmul(avs[:], Vpp[h // 2][:, J, vsl],
                                     atT[:, js],
                                     start=(J == 0), stop=(J == NT - 1))
                prt = slice((h % 2) * 64, (h % 2) * 64 + 64)
                nc.vector.tensor_copy(OT[h // 2][prt, isl], avs[0:64, :])
                nc.vector.tensor_copy(OT[2 + h // 2][prt, isl],
                                      avs[64:128, :])

            def emit_outproj_unit(nh, part, dt_):
                ns = slice(nh * 512, (nh + 1) * 512)
                wo_t, bo_t = (wo_sr, bo_rt) if part == "r" else (wo_si, bo_it)
                if True:
                    if True:
                        ds = slice(dt_ * 128, (dt_ + 1) * 128)
                        ops = psC.tile([128, 512], F32, tag="pc",
                                       name=f"ops_{part}_{dt_}_{nh}")
                        for j in range(4):
                            nc.tensor.matmul(ops[:], wo_t[:, j, ds],
                                             OT[j][:, ns],
                                             start=(j == 0), stop=(j == 3))
                        osb = pout.tile([128, 512], F32, tag="ot",
                                        name=f"osb_{part}_{dt_}_{nh}")
                        if nh == 1:
                            # drain tail: DVE empties ~16us before ACT here
                            nc.vector.tensor_scalar_add(osb[:], ops[:],
                                                        bo_t[:, dt_:dt_ + 1])
                        else:
                            nc.scalar.activation(osb[:], ops[:], AF.Identity,
                                                 bias=bo_t[:, dt_:dt_ + 1])
                        dst = o_r if part == "r" else o_i
                        nc.sync.dma_start(
                            bass.AP(dst, dt_ * 128 * N + nh * 512,
                                    [[N, 128], [1, 512]]),
                            osb[:])

            def emit_outproj(nh):
                for part in ("r", "i"):
                    for dt_ in range(4):
                        emit_outproj_unit(nh, part, dt_)

            ounits = []

            flat = [(h, I) for h in range(HPC) for I in range(NT)]
            PB2, PC, PD = PB + 1, PB + 2, PB + 3
            rmap, skewmap, bmap, b2map, cmap = {}, {}, {}, {}, {}
            # late stages are emitted FIRST within each step so that
            # dependency waits of the stage-B tail never head-of-line block
            # the in-order engine queues for already-runnable work
            for s in range(len(flat) + PD + 1):
                for _ in range(2):
                    if units:
                        u = units.pop(0)
                        if u[0] == "v":
                            emit_vproj(u[1], u[2])
                        else:
                            emit_proj(*u)
                if s < len(flat):
                    h, I = flat[s]
                    rmap[(h, I)] = emit_qrel_r(h, I)
                if 1 <= s < len(flat) + 1:
                    h, I = flat[s - 1]
                    skewmap[(h, I)] = emit_qrel_i(h, I, rmap.pop((h, I)))
                if PB <= s < len(flat) + PB:
                    h, I = flat[s - PB]
                    bmap[(h, I)] = stage_B(h, I, skewmap.pop((h, I)))
                if PB2 <= s < len(flat) + PB2:
                    h, I = flat[s - PB2]
                    b2map[(h, I)] = stage_B2(h, I, bmap.pop((h, I)))
                if PC <= s < len(flat) + PC:
                    h, I = flat[s - PC]
                    cmap[(h, I)] = stage_C(h, I, b2map.pop((h, I)))
                for _ in range(2):
                    if ounits:
                        emit_outproj_unit(*ounits.pop(0))
                if PD <= s < len(flat) + PD:
                    h, I = flat[s - PD]
                    stage_D(h, I, cmap.pop((h, I)))
                    if (h, I) == (HPC - 1, 3):
                        ounits.extend((0, part, dt_) for part in ("r", "i")
                                      for dt_ in range(4))
            while ounits:
                emit_outproj_unit(*ounits.pop(0))
            emit_outproj(1)

    nc.compile()
    return nc, addsq


def _prep_core_inputs(inputs, core):
    b, half = core // 2, core % 2
    x = inputs["x"]
    f16 = np.float16
    f32 = np.float32
    xt_r = np.ascontiguousarray(x[b, :, :, 0].T).astype(f16)
    xt_i = np.ascontiguousarray(x[b, :, :, 1].T).astype(f16)

    def pack_ab(wr, wi):
        a = np.empty((DIM, 512), f32)
        bb = np.empty((DIM, 512), f32)
        for hl in range(HPC):
            gh = half * HPC + hl
            cs = slice(gh * DH, (gh + 1) * DH)
            a[:, hl * 128:hl * 128 + 64] = wr[:, cs]
            a[:, hl * 128 + 64:hl * 128 + 128] = wi[:, cs]
            bb[:, hl * 128:hl * 128 + 64] = -wi[:, cs]
            bb[:, hl * 128 + 64:hl * 128 + 128] = wr[:, cs]
        return a.astype(f16), bb.astype(f16)

    wq_a, wq_b = pack_ab(inputs["wq_r"], inputs["wq_i"])
    wk_a, wk_b = pack_ab(inputs["wkv_r"][:, :512], inputs["wkv_i"][:, :512])
    wv_a, wv_b = pack_ab(inputs["wkv_r"][:, 512:], inputs["wkv_i"][:, 512:])

    rs = slice(half * 256, (half + 1) * 256)
    wo_sr = np.concatenate(
        [inputs["wo_r"][rs, :], -inputs["wo_i"][rs, :]], 0).astype(f16)
    wo_si = np.concatenate(
        [inputs["wo_i"][rs, :], inputs["wo_r"][rs, :]], 0).astype(f16)

    e = np.arange(2047)
    t_ext = inputs["rel_emb"][np.clip(e - 1023, -MAX_POS, MAX_POS) + MAX_POS]
    relrev = t_ext[::-1].astype(f32)           # [2047, 64]
    rel_r = np.zeros((128, 2048), f32)
    rel_i = np.zeros((128, 2048), f32)
    rel_r[0:64, 0:2047] = relrev.T
    rel_i[64:128, 0:2047] = -relrev.T

    bscale = 1.0 if half == 0 else 0.0
    bo_rt = np.ascontiguousarray(
        inputs["bo_r"].reshape(4, 128).T * bscale).astype(f32)
    bo_it = np.ascontiguousarray(
        inputs["bo_i"].reshape(4, 128).T * bscale).astype(f32)
    smask = np.concatenate(
        [np.full(64, SCALE, f32), np.full(64, -SCALE, f32)]).reshape(128, 1)

    return {
        "xt_r": xt_r, "xt_i": xt_i,
        "wq_a": wq_a, "wq_b": wq_b, "wk_a": wk_a, "wk_b": wk_b,
        "wv_a": wv_a, "wv_b": wv_b,
        "wo_sr": wo_sr, "wo_si": wo_si,
        "rel_r": rel_r.astype(f16), "rel_i": rel_i.astype(f16),
        "bo_rt": bo_rt, "bo_it": bo_it, "smask": smask,
    }


_last_results = {}


def kernel(**inputs):
    inputs = {k: np.asarray(v) for k, v in inputs.items()}
    nc, _ = build_module()
    in_maps = [_prep_core_inputs(inputs, c) for c in range(8)]
    res = run_bass_kernel_spmd(nc, in_maps, core_ids=list(range(8)))
    _last_results["res"] = res

    out = np.empty((B, N, DIM, 2), np.float32)
    for b in range(B):
        r = res.results[2 * b]["o_r"] + res.results[2 * b + 1]["o_r"]
        i = res.results[2 * b]["o_i"] + res.results[2 * b + 1]["o_i"]
        out[b, :, :, 0] = r.T
        out[b, :, :, 1] = i.T
    return out


# revision 52
# speedup vs baseline: 1.0293x; 1.0001x over previous
"""Complex-valued relative-position attention (nn_CAttention) on 8 TRN2 cores.

Sharding: batch (4) x head-half (2) -> 8 cores. Each core computes its
batch's projections for its 4 heads, full attention for those heads, and a
row-split partial output projection. Host sums the two partial outputs per
batch and restacks.

v2 redesign (engine rebalance, f16 matmul inputs):
  - All matmul moving operands f16 (moving-side dtype sets the PE rate:
    1 cyc/row at any N; baseline's f32r qrel tail chunks ran at 4 cyc/row).
  - Half-inject: the i-part skew lands in the dots_i PSUM group via an
    extra identity-matmul accumulation (PE), so stage-B DVE work is two
    fused custom ops: er=(skw_r+dots_r)^2 (ADDSQ) and m2=er+dots_i^2
    (SQACC), instead of 2x ADDSQ + add.
  - qrel r/i parts pipeline through ONE f32 PSUM slot on alternating
    steps (r at step s, i at step s+1), with the PSUM->SBUF f16 copies
    split: r on ACT, i on DVE, 128-wide tails on Pool.
  - One combined skew DMA write [128,2304] and one 3-level-AP read
    [128,2048] per (h,I), both on the SP HWDGE queue (engine-free).
  - to_out negation folded into host-packed weights ([wo_r; -wo_i] /
    [wo_i; wo_r] row stacks): stage_D needs only 2 PSUM->OT copies (Pool).
  - attn transpose copy (atT) and V copies on Pool; ln/exp/exp softmax
    chain on ACT (pinned ln+exp table set, zero table switches).
"""
import functools
import numpy as np

import concourse.bass as bass
import concourse.bacc as bacc
import concourse.mybir as mybir
import concourse.tile as tile
from concourse.bass_utils import run_bass_kernel_spmd
from concourse.masks import make_identity

F32 = mybir.dt.float32
F16 = mybir.dt.float16
AF = mybir.ActivationFunctionType
ALU = mybir.AluOpType
U16 = mybir.dt.uint16

HEADS, DH, MAX_POS = 8, 64, 512
B, N, DIM = 4, 1024, 512
HPC = 4            # heads per core
KT = 4             # dim k-tiles (512/128)
NT = 8             # n tiles (1024/128)
SCALE = DH ** (-0.5)
PB = 3             # qrel->stage_B pipeline distance (iterations)


def register_custom():
    from concourse import dve_ops
    from concourse.dve_spec import Spec, Src0, Src1, AluOp, Bin, lower, sq
    from concourse.dve_spec import C0, C1
    from concourse.dve_uop import DveOpSpec

    def mk(name, body, ref):
        for op in dve_ops.OPS:
            if op.name == name:
                return op
        spec = Spec(body=body, reference=ref)
        opcode = dve_ops._CUSTOM_DVE_ROW_BASE + len(dve_ops.OPS)
        shas = {}
        for ver in ("v3",):
            s = DveOpSpec(name=name, opcode=opcode,
                          uops=lower(spec, ver=ver), rd1_en=True)
            shas[ver] = s.sha(ver)
        op = dve_ops.DveOp(name, spec, subdim=False, uops_sha=shas)
        dve_ops._SUB_OPCODE_FOR_NAME[op.name] = opcode
        dve_ops.OPS.append(op)
        dve_ops.CUSTOM_DVE_SPECS[op.name] = op.spec
        return op

    addsq = mk(
        "ADDSQ_ANT",
        sq(Bin(AluOp.ADD, Src0, Src1)),
        lambda in0, in1, s0, s1, imm2: (
            (in0.astype(np.float32) + in1.astype(np.float32)) ** 2),
    )
    sqacc = mk(
        "SQACC_ANT",
        Bin(AluOp.ADD, Src0, sq(Src1)),
        lambda in0, in1, s0, s1, imm2: (
            in0.astype(np.float32) + in1.astype(np.float32) ** 2),
    )
    # m = x*r0*(C0 - C1*(x*r0)*r0): one Newton-Raphson rsqrt step folded
    # with the final multiply; Src0 = x (=m2), Src1 = r0 (bit-trick rsqrt
    # seed), C0 = 1.5, C1 = 0.5. Produces sqrt(x) to ~0.2%.
    _t = Bin(AluOp.MULTIPLY, Src0, Src1)
    nsqrt = mk(
        "NSQRT_ANT",
        Bin(AluOp.MULTIPLY, _t,
            Bin(AluOp.SUBTRACT, C0,
                Bin(AluOp.MULTIPLY, C1,
                    Bin(AluOp.MULTIPLY, _t, Src1)))),
        lambda in0, in1, s0, s1, imm2: (
            (in0.astype(np.float32) * in1.astype(np.float32))
            * (s0 - s1 * (in0.astype(np.float32) * in1.astype(np.float32))
               * in1.astype(np.float32))),
    )
    return addsq, sqacc, nsqrt


def c_lo(i_blk):
    return 896 - 128 * i_blk


@functools.cache
def build_module():
    import concourse.tile_utils as tile_utils
    if getattr(tile_utils, "max_sbuf_usage", 0) < 208 * 1024:
        tile_utils.max_sbuf_usage = 208 * 1024

    # Pin the ACT engine to the ln+exp table set: every activation this
    # kernel emits (Ln, Exp, Copy/Identity) lives in that one set, so the
    # greedy table-load pass emits exactly one LoadActFuncSet.
    import concourse.bacc as bacc_mod
    if not getattr(bacc_mod, "_ant_act_tables_pinned", False):
        orig_gat = bacc_mod.get_activation_tables

        def pinned_gat(arch):
            full = orig_gat(arch)
            out = {}
            for name, funcs in full.items():
                if name != "natural_log_exp_and_others":
                    funcs = funcs - {mybir.ActivationFunctionType.Ln,
                                     mybir.ActivationFunctionType.Exp}
                out[name] = funcs
            return out

        bacc_mod.get_activation_tables = pinned_gat
        bacc_mod._ant_act_tables_pinned = True

    addsq, sqacc, nsqrt = register_custom()
    nc = bacc.Bacc("TRN2", target_bir_lowering=False, debug=False,
                   num_devices=8)

    din = {}
    for nm, shape, dt_ in [
        ("xt_r", [DIM, N], F16), ("xt_i", [DIM, N], F16),
        ("wq_a", [DIM, 512], F16), ("wq_b", [DIM, 512], F16),
        ("wk_a", [DIM, 512], F16), ("wk_b", [DIM, 512], F16),
        ("wv_a", [DIM, 512], F16), ("wv_b", [DIM, 512], F16),
        ("wo_sr", [DIM, 512], F16), ("wo_si", [DIM, 512], F16),
        ("rel_r", [128, 2048], F16), ("rel_i", [128, 2048], F16),
        ("bo_rt", [128, 4], F32), ("bo_it", [128, 4], F32),
        ("smask", [128, 1], F32),
    ]:
        din[nm] = nc.dram_tensor(nm, shape, dt_, kind="ExternalInput")
    o_r = nc.dram_tensor("o_r", [DIM, N], F32, kind="ExternalOutput")
    o_i = nc.dram_tensor("o_i", [DIM, N], F32, kind="ExternalOutput")

    with tile.TileContext(nc) as tc:
        with (
            tc.tile_pool(name="const", bufs=1) as cpool,
            tc.tile_pool(name="wts", bufs=6) as pwt,      # 4KB (weights)
            tc.tile_pool(name="xts", bufs=8) as pxt,      # 2KB (x tiles)
            tc.tile_pool(name="work", bufs=28) as pw,     # 2KB slots
            tc.tile_pool(name="qev", bufs=4) as pqe,      # 4.5KB slots
            tc.tile_pool(name="skew", bufs=6) as psk,     # 4KB slots
            tc.tile_pool(name="stacks", bufs=16) as pstk,  # 2KB slots
            tc.tile_pool(name="vstk", bufs=2) as pvp,     # 4KB slots
            tc.tile_pool(name="outsb", bufs=6) as pout,
            tc.tile_pool(name="small", bufs=16) as psm,
            tc.tile_pool(name="psD", bufs=3, space="PSUM") as psD,   # 2-bank
            tc.tile_pool(name="psC", bufs=2, space="PSUM") as psC,   # 1-bank
            tc.tile_pool(name="dram", bufs=7, space="DRAM") as pdram,
        ):
            # ---------------- constants ----------------
            id16 = cpool.tile([128, 128], F16, tag="id16")
            make_identity(nc, id16[:])
            smask = cpool.tile([128, 1], F32, tag="smask")
            nc.sync.dma_start(smask[:], din["smask"][:, :])

            def load_w4(nm):
                # [512, 512] dram -> [128, 4, 512] tile (one DMA)
                t = pwt.tile([128, 4, 512], F16, tag="w4", name=nm)
                nc.sync.dma_start(
                    t[:],
                    bass.AP(din[nm], 0, [[512, 128], [128 * 512, 4], [1, 512]]))
                return t


            # ---------------- phase P: projections ----------------
            A = [None] * HPC
            A2 = [None] * HPC
            Knat = [None] * HPC

            wq = (load_w4("wq_a"), load_w4("wq_b"))

            # xt tiles: [128, 1024] per (r/i, kt)
            xt = {}
            for nm in ("xt_r", "xt_i"):
                for kt in range(KT):
                    t = pxt.tile([128, 1024], F16, tag="xt",
                                 name=f"{nm}_{kt}")
                    nc.sync.dma_start(
                        t[:], bass.AP(din[nm], kt * 128 * N,
                                      [[N, 128], [1, 1024]]))
                    xt[(nm, kt)] = t

            wk = (load_w4("wk_a"), load_w4("wk_b"))
            wva = load_w4("wv_a")
            wvb = load_w4("wv_b")

            rel_r = cpool.tile([128, 2048], F16, tag="rel_r")
            rel_i = cpool.tile([128, 2048], F16, tag="rel_i")
            nc.sync.dma_start(rel_r[:], din["rel_r"][:, :])
            nc.sync.dma_start(rel_i[:], din["rel_i"][:, :])
            bo_rt = cpool.tile([128, 4], F32, tag="bo_rt")
            bo_it = cpool.tile([128, 4], F32, tag="bo_it")
            nc.sync.dma_start(bo_rt[:], din["bo_rt"][:, :])
            nc.sync.dma_start(bo_it[:], din["bo_it"][:, :])
            wo_sr = cpool.tile([128, 4, 512], F16, tag="wo_sr")
            wo_si = cpool.tile([128, 4, 512], F16, tag="wo_si")
            nc.sync.dma_start(
                wo_sr[:],
                bass.AP(din["wo_sr"], 0, [[512, 128], [128 * 512, 4], [1, 512]]))
            nc.sync.dma_start(
                wo_si[:],
                bass.AP(din["wo_si"], 0, [[512, 128], [128 * 512, 4], [1, 512]]))

            def emit_proj(kind, h, nh):
                wa, wb = wq if kind == "q" else wk
                hs = slice(h * 128, (h + 1) * 128)
                if nh == 0:
                    if kind == "q":
                        A[h] = pstk.tile([128, 1024], F16, tag="stk",
                                         name=f"A{h}")
                    else:
                        Knat[h] = pstk.tile([128, 1024], F16, tag="stk",
                                            name=f"Knat{h}")
                ns = slice(nh * 512, (nh + 1) * 512)
                ps = psD.tile([128, 512], F32, tag="pd",
                              name=f"ps{kind}_{h}_{nh}")
                for kt in range(KT):
                    nc.tensor.matmul(ps[:], wa[:, kt, hs],
                                     xt[("xt_r", kt)][:, ns],
                                     start=(kt == 0), stop=False)
                for kt in range(KT):
                    nc.tensor.matmul(ps[:], wb[:, kt, hs],
                                     xt[("xt_i", kt)][:, ns],
                                     start=False, stop=(kt == KT - 1))
                if kind == "q":
                    nc.scalar.mul(A[h][:, ns], ps[:], smask[:])
                else:
                    nc.scalar.copy(Knat[h][:, ns], ps[:])
                if kind == "q" and nh == 1:
                    # A2 = [s*qi; s*qr] built from A = [s*qr; -s*qi] on
                    # Pool (SBUF->SBUF), so dots_i = A2^T @ Knat and the
                    # Kni2 stack is never materialized
                    A2[h] = pstk.tile([128, 1024], F16, tag="stk",
                                      name=f"A2_{h}")
                    nc.gpsimd.tensor_scalar_mul(A2[h][0:64, :],
                                                A[h][64:128, :], -1.0)
                    nc.gpsimd.tensor_copy(A2[h][64:128, :], A[h][0:64, :])

            for kind, h in (("q", 0), ("q", 1), ("k", 0)):
                for nh in range(2):
                    emit_proj(kind, h, nh)
            Vpp = [pvp.tile([128, 8, 256], F16, tag="vs", name=f"Vpp{p}")
                   for p in range(2)]

            def emit_vproj(p, J):
                # two heads per matmul (256-wide output)
                hs = slice(p * 256, (p + 1) * 256)
                js = slice(J * 128, (J + 1) * 128)
                vps = psC.tile([128, 256], F32, tag="pc",
                               name=f"vps_{p}_{J}")
                for kt in range(KT):
                    nc.tensor.matmul(vps[:],
                                     xt[("xt_r", kt)][:, js],
                                     wva[:, kt, hs],
                                     start=(kt == 0), stop=False)
                for kt in range(KT):
                    nc.tensor.matmul(vps[:],
                                     xt[("xt_i", kt)][:, js],
                                     wvb[:, kt, hs],
                                     start=False, stop=(kt == KT - 1))
                nc.scalar.copy(Vpp[p][:, J, :], vps[:])

            units = ([("k", 1, 0), ("k", 1, 1)]
                     + [("v", 0, J) for J in range(NT)]
                     + [(k, h, nh) for h in (2, 3) for k in ("q", "k")
                        for nh in range(2)]
                     + [("v", 1, J) for J in range(NT)])

            # OT stacks for the output projection:
            # OT[0] = or heads 0,1 | OT[1] = or heads 2,3
            # OT[2] = oi heads 0,1 | OT[3] = oi heads 2,3
            OT = [pstk.tile([128, 1024], F16, tag="stk", name=f"OT{t}")
                  for t in range(4)]

            # ---------------- phase A: pipelined attention ----------------
            # Per (h, I): qrel r-part at step s, i-part + skew write/read at
            # step s+1, dots + softmax at s+PB, transpose at s+PB+2, AV at
            # s+PB+3.
            def emit_qrel_r(h, I):
                isl = slice(I * 128, (I + 1) * 128)
                lo = c_lo(I)
                qe = pqe.tile([128, 2, 1152], F16, tag="qe",
                              name=f"qe_{h}_{I}")
                qtl = psC.tile([128, 2, 128], F32, tag="pc",
                               name=f"qtl_{h}_{I}")
                qps = psD.tile([128, 1024], F32, tag="pd",
                               name=f"qpsr_{h}_{I}")
                for c0 in (0, 512):
                    nc.tensor.matmul(qps[:, c0:c0 + 512], A[h][:, isl],
                                     rel_r[:, lo + c0:lo + c0 + 512],
                                     start=True, stop=True)
                nc.tensor.matmul(qtl[:, 0, :], A[h][:, isl],
                                 rel_r[:, lo + 1024:lo + 1152],
                                 start=True, stop=True)
                nc.scalar.copy(qe[:, 0, 0:1024], qps[:])
                return {"qe": qe, "qtl": qtl}

            def emit_qrel_i(h, I, st):
                isl = slice(I * 128, (I + 1) * 128)
                lo = c_lo(I)
                qe, qtl = st["qe"], st["qtl"]
                qps = psD.tile([128, 1024], F32, tag="pd",
                               name=f"qpsi_{h}_{I}")
                for c0 in (0, 512):
                    nc.tensor.matmul(qps[:, c0:c0 + 512], A[h][:, isl],
                                     rel_i[:, lo + c0:lo + c0 + 512],
                                     start=True, stop=True)
                nc.tensor.matmul(qtl[:, 1, :], A[h][:, isl],
                                 rel_i[:, lo + 1024:lo + 1152],
                                 start=True, stop=True)
                nc.scalar.copy(qe[:, 1, 0:1024], qps[:])
                nc.scalar.copy(qe[:, :, 1024:1152], qtl[:])
                slot = pdram.tile([128, 2304], F16, tag="qrev",
                                  name=f"qrev_{h}_{I}")
                nc.sync.dma_start(slot[:, :], qe[:])
                skw = psk.tile([128, 2048], F16, tag="skw",
                               name=f"skew_{h}_{I}")
                nc.sync.dma_start(
                    skw[:],
                    bass.AP(slot.tensor, 127,
                            [[2303, 128], [1152, 2], [1, 1024]]))
                return skw

            def stage_B(h, I, skw):
                isl = slice(I * 128, (I + 1) * 128)
                dpsr = psD.tile([128, 1024], F32, tag="pd",
                                name=f"dpsr_{h}_{I}")
                dpsi = psD.tile([128, 1024], F32, tag="pd",
                                name=f"dpsi_{h}_{I}")
                # i-part skew injected into the dots_i PSUM group via
                # identity-matmul accumulation (DVE can read only one PSUM
                # operand per op, so r-part adds via the fused ADDSQ)
                for nh in range(2):
                    ns = slice(nh * 512, (nh + 1) * 512)
                    nc.tensor.matmul(dpsr[:, ns], A[h][:, isl],
                                     Knat[h][:, ns], start=True, stop=True)
                    nc.tensor.matmul(dpsi[:, ns], A2[h][:, isl],
                                     Knat[h][:, ns], start=True, stop=False)
                    nc.tensor.matmul(dpsi[:, ns], id16[:],
                                     skw[:, 1024 + nh * 512:1024 + nh * 512 + 512],
                                     start=False, stop=True)
                er = pw.tile([128, 1024], F16, tag="wk", name=f"er_{h}_{I}")
                nc.vector._custom_dve(addsq, out=er[:],
                                      in0=skw[:, 0:1024], in1=dpsr[:])
                m2 = pw.tile([128, 1024], F16, tag="wk", name=f"m2_{h}_{I}")
                nc.vector._custom_dve(sqacc, out=m2[:],
                                      in0=er[:], in1=dpsi[:])
                # sqrt(m2) without the ACT ln/exp round trip: integer-view
                # rsqrt bit-seed on Pool, then one fused Newton step * m2 on
                # DVE (NSQRT, next pipeline step).  bits(r0) = 22971 -
                # bits(m2)/2.
                r0 = pw.tile([128, 1024], F16, tag="wk", name=f"r0_{h}_{I}")
                nc.gpsimd.tensor_scalar(
                    r0[:].bitcast(U16), m2[:].bitcast(U16),
                    -0.5, 22971.0, ALU.mult, ALU.add)
                return {"m2": m2, "r0": r0}

            def stage_B2(h, I, st):
                m2, r0 = st["m2"], st["r0"]
                mt = pw.tile([128, 1024], F16, tag="wk", name=f"mt_{h}_{I}")
                nc.vector._custom_dve(nsqrt, out=mt[:],
                                      in0=m2[:], in1=r0[:],
                                      s0=1.5, s1=0.5)
                attn = pw.tile([128, 1024], F16, tag="wk",
                               name=f"attn_{h}_{I}")
                rs = psm.tile([128, 1], F32, tag="sm", name=f"rs_{h}_{I}")
                nc.scalar.activation(attn[:], mt[:], AF.Exp, accum_out=rs[:])
                rc = psm.tile([128, 1], F32, tag="sm", name=f"rc_{h}_{I}")
                nc.vector.reciprocal(rc[:], rs[:])
                # normalize on Pool (SBUF-only op), a full step ahead of the
                # transposes that consume attn
                nc.gpsimd.tensor_scalar_mul(attn[:], attn[:], rc[:])
                return {"attn": attn}

            def stage_C(h, I, st):
                attn = st["attn"]
                tps = psC.tile([128, 1024], F16, tag="pc", name=f"tps_{h}_{I}")
                for J in range(NT):
                    js = slice(J * 128, (J + 1) * 128)
                    nc.tensor.transpose(tps[:, js], attn[:, js], id16[:])
                atT = pw.tile([128, 1024], F16, tag="wk", name=f"atT_{h}_{I}")
                nc.vector.tensor_copy(atT[:], tps[:])
                return atT

            def stage_D(h, I, atT):
                isl = slice(I * 128, (I + 1) * 128)
                avs = psC.tile([128, 128], F32, tag="pc", name=f"avs_{h}_{I}")
                vsl = slice((h % 2) * 128, (h % 2) * 128 + 128)
                for J in range(NT):
                    js = slice(J * 128, (J + 1) * 128)
                    nc.tensor.matmul(avs[:], Vpp[h // 2][:, J, vsl],
                                     atT[:, js],
                                     start=(J == 0), stop=(J == NT - 1))
                prt = slice((h % 2) * 64, (h % 2) * 64 + 64)
                nc.vector.tensor_copy(OT[h // 2][prt, isl], avs[0:64, :])
                nc.vector.tensor_copy(OT[2 + h // 2][prt, isl],
                                      avs[64:128, :])

            def emit_outproj_unit(nh, part, dt_):
                ns = slice(nh * 512, (nh + 1) * 512)
                wo_t, bo_t = (wo_sr, bo_rt) if part == "r" else (wo_si, bo_it)
                if True:
                    if True:
                        ds = slice(dt_ * 128, (dt_ + 1) * 128)
                        ops = psC.tile([128, 512], F32, tag="pc",
                                       name=f"ops_{part}_{dt_}_{nh}")
                        for j in range(4):
                            nc.tensor.matmul(ops[:], wo_t[:, j, ds],
                                             OT[j][:, ns],
                                             start=(j == 0), stop=(j == 3))
                        osb = pout.tile([128, 512], F32, tag="ot",
                                        name=f"osb_{part}_{dt_}_{nh}")
                        if nh == 1:
                            # drain tail: DVE empties ~16us before ACT here
                            nc.vector.tensor_scalar_add(osb[:], ops[:],
                                                        bo_t[:, dt_:dt_ + 1])
                        else:
                            nc.scalar.activation(osb[:], ops[:], AF.Identity,
                                                 bias=bo_t[:, dt_:dt_ + 1])
                        dst = o_r if part == "r" else o_i
                        nc.sync.dma_start(
                            bass.AP(dst, dt_ * 128 * N + nh * 512,
                                    [[N, 128], [1, 512]]),
                            osb[:])

            def emit_outproj(nh):
                for part in ("r", "i"):
                    for dt_ in range(4):
                        emit_outproj_unit(nh, part, dt_)

            ounits = []

            flat = [(h, I) for h in range(HPC) for I in range(NT)]
            PB2, PC, PD = PB + 1, PB + 2, PB + 3
            rmap, skewmap, bmap, b2map, cmap = {}, {}, {}, {}, {}
            # late stages are emitted FIRST within each step so that
            # dependency waits of the stage-B tail never head-of-line block
            # the in-order engine queues for already-runnable work
            for s in range(len(flat) + PD + 1):
                for _ in range(2):
                    if units:
                        u = units.pop(0)
                        if u[0] == "v":
                            emit_vproj(u[1], u[2])
                        else:
                            emit_proj(*u)
                if s < len(flat):
                    h, I = flat[s]
                    rmap[(h, I)] = emit_qrel_r(h, I)
                if 1 <= s < len(flat) + 1:
                    h, I = flat[s - 1]
                    skewmap[(h, I)] = emit_qrel_i(h, I, rmap.pop((h, I)))
                if PB <= s < len(flat) + PB:
                    h, I = flat[s - PB]
                    bmap[(h, I)] = stage_B(h, I, skewmap.pop((h, I)))
                if PB2 <= s < len(flat) + PB2:
                    h, I = flat[s - PB2]
                    b2map[(h, I)] = stage_B2(h, I, bmap.pop((h, I)))
                if PC <= s < len(flat) + PC:
                    h, I = flat[s - PC]
                    cmap[(h, I)] = stage_C(h, I, b2map.pop((h, I)))
                for _ in range(3):
                    if ounits:
                        emit_outproj_unit(*ounits.pop(0))
                if PD <= s < len(flat) + PD:
                    h, I = flat[s - PD]
                    stage_D(h, I, cmap.pop((h, I)))
                    if (h, I) == (HPC - 1, 3):
                        ounits.extend((0, part, dt_) for part in ("r", "i")
                                      for dt_ in range(4))
            while ounits:
                emit_outproj_unit(*ounits.pop(0))
            emit_outproj(1)

    nc.compile()
    return nc, addsq


def _prep_core_inputs(inputs, core):
    b, half = core // 2, core % 2
    x = inputs["x"]
    f16 = np.float16
    f32 = np.float32
    xt_r = np.ascontiguousarray(x[b, :, :, 0].T).astype(f16)
    xt_i = np.ascontiguousarray(x[b, :, :, 1].T).astype(f16)

    def pack_ab(wr, wi):
        a = np.empty((DIM, 512), f32)
        bb = np.empty((DIM, 512), f32)
        for hl in range(HPC):
            gh = half * HPC + hl
            cs = slice(gh * DH, (gh + 1) * DH)
            a[:, hl * 128:hl * 128 + 64] = wr[:, cs]
            a[:, hl * 128 + 64:hl * 128 + 128] = wi[:, cs]
            bb[:, hl * 128:hl * 128 + 64] = -wi[:, cs]
            bb[:, hl * 128 + 64:hl * 128 + 128] = wr[:, cs]
        return a.astype(f16), bb.astype(f16)

    wq_a, wq_b = pack_ab(inputs["wq_r"], inputs["wq_i"])
    wk_a, wk_b = pack_ab(inputs["wkv_r"][:, :512], inputs["wkv_i"][:, :512])
    wv_a, wv_b = pack_ab(inputs["wkv_r"][:, 512:], inputs["wkv_i"][:, 512:])

    rs = slice(half * 256, (half + 1) * 256)
    wo_sr = np.concatenate(
        [inputs["wo_r"][rs, :], -inputs["wo_i"][rs, :]], 0).astype(f16)
    wo_si = np.concatenate(
        [inputs["wo_i"][rs, :], inputs["wo_r"][rs, :]], 0).astype(f16)

    e = np.arange(2047)
    t_ext = inputs["rel_emb"][np.clip(e - 1023, -MAX_POS, MAX_POS) + MAX_POS]
    relrev = t_ext[::-1].astype(f32)           # [2047, 64]
    rel_r = np.zeros((128, 2048), f32)
    rel_i = np.zeros((128, 2048), f32)
    rel_r[0:64, 0:2047] = relrev.T
    rel_i[64:128, 0:2047] = -relrev.T

    bscale = 1.0 if half == 0 else 0.0
    bo_rt = np.ascontiguousarray(
        inputs["bo_r"].reshape(4, 128).T * bscale).astype(f32)
    bo_it = np.ascontiguousarray(
        inputs["bo_i"].reshape(4, 128).T * bscale).astype(f32)
    smask = np.concatenate(
        [np.full(64, SCALE, f32), np.full(64, -SCALE, f32)]).reshape(128, 1)

    return {
        "xt_r": xt_r, "xt_i": xt_i,
        "wq_a": wq_a, "wq_b": wq_b, "wk_a": wk_a, "wk_b": wk_b,
        "wv_a": wv_a, "wv_b": wv_b,
        "wo_sr": wo_sr, "wo_si": wo_si,
        "rel_r": rel_r.astype(f16), "rel_i": rel_i.astype(f16),
        "bo_rt": bo_rt, "bo_it": bo_it, "smask": smask,
    }


_last_results = {}


def kernel(**inputs):
    inputs = {k: np.asarray(v) for k, v in inputs.items()}
    nc, _ = build_module()
    in_maps = [_prep_core_inputs(inputs, c) for c in range(8)]
    res = run_bass_kernel_spmd(nc, in_maps, core_ids=list(range(8)))
    _last_results["res"] = res

    out = np.empty((B, N, DIM, 2), np.float32)
    for b in range(B):
        r = res.results[2 * b]["o_r"] + res.results[2 * b + 1]["o_r"]
        i = res.results[2 * b]["o_i"] + res.results[2 * b + 1]["o_i"]
        out[b, :, :, 0] = r.T
        out[b, :, :, 1] = i.T
    return out


# revision 53
# speedup vs baseline: 1.0504x; 1.0205x over previous
"""Complex-valued relative-position attention (nn_CAttention) on 8 TRN2 cores.

Sharding: batch (4) x head-half (2) -> 8 cores. Each core computes its
batch's projections for its 4 heads, full attention for those heads, and a
row-split partial output projection. Host sums the two partial outputs per
batch and restacks.

v2 redesign (engine rebalance, f16 matmul inputs):
  - All matmul moving operands f16 (moving-side dtype sets the PE rate:
    1 cyc/row at any N; baseline's f32r qrel tail chunks ran at 4 cyc/row).
  - Half-inject: the i-part skew lands in the dots_i PSUM group via an
    extra identity-matmul accumulation (PE), so stage-B DVE work is two
    fused custom ops: er=(skw_r+dots_r)^2 (ADDSQ) and m2=er+dots_i^2
    (SQACC), instead of 2x ADDSQ + add.
  - qrel r/i parts pipeline through ONE f32 PSUM slot on alternating
    steps (r at step s, i at step s+1), with the PSUM->SBUF f16 copies
    split: r on ACT, i on DVE, 128-wide tails on Pool.
  - One combined skew DMA write [128,2304] and one 3-level-AP read
    [128,2048] per (h,I), both on the SP HWDGE queue (engine-free).
  - to_out negation folded into host-packed weights ([wo_r; -wo_i] /
    [wo_i; wo_r] row stacks): stage_D needs only 2 PSUM->OT copies (Pool).
  - attn transpose copy (atT) and V copies on Pool; ln/exp/exp softmax
    chain on ACT (pinned ln+exp table set, zero table switches).
"""
import functools
import numpy as np

import concourse.bass as bass
import concourse.bacc as bacc
import concourse.mybir as mybir
import concourse.tile as tile
from concourse.bass_utils import run_bass_kernel_spmd
from concourse.masks import make_identity

F32 = mybir.dt.float32
F16 = mybir.dt.float16
AF = mybir.ActivationFunctionType
ALU = mybir.AluOpType
U16 = mybir.dt.uint16

HEADS, DH, MAX_POS = 8, 64, 512
B, N, DIM = 4, 1024, 512
HPC = 4            # heads per core
KT = 4             # dim k-tiles (512/128)
NT = 8             # n tiles (1024/128)
SCALE = DH ** (-0.5)
PB = 3             # qrel->stage_B pipeline distance (iterations)


def register_custom():
    from concourse import dve_ops
    from concourse.dve_spec import Spec, Src0, Src1, AluOp, Bin, lower, sq
    from concourse.dve_spec import C0, C1
    from concourse.dve_uop import DveOpSpec

    def mk(name, body, ref):
        for op in dve_ops.OPS:
            if op.name == name:
                return op
        spec = Spec(body=body, reference=ref)
        opcode = dve_ops._CUSTOM_DVE_ROW_BASE + len(dve_ops.OPS)
        shas = {}
        for ver in ("v3",):
            s = DveOpSpec(name=name, opcode=opcode,
                          uops=lower(spec, ver=ver), rd1_en=True)
            shas[ver] = s.sha(ver)
        op = dve_ops.DveOp(name, spec, subdim=False, uops_sha=shas)
        dve_ops._SUB_OPCODE_FOR_NAME[op.name] = opcode
        dve_ops.OPS.append(op)
        dve_ops.CUSTOM_DVE_SPECS[op.name] = op.spec
        return op

    addsq = mk(
        "ADDSQ_ANT",
        sq(Bin(AluOp.ADD, Src0, Src1)),
        lambda in0, in1, s0, s1, imm2: (
            (in0.astype(np.float32) + in1.astype(np.float32)) ** 2),
    )
    sqacc = mk(
        "SQACC_ANT",
        Bin(AluOp.ADD, Src0, sq(Src1)),
        lambda in0, in1, s0, s1, imm2: (
            in0.astype(np.float32) + in1.astype(np.float32) ** 2),
    )
    # m = x*r0*(C0 - C1*(x*r0)*r0): one Newton-Raphson rsqrt step folded
    # with the final multiply; Src0 = x (=m2), Src1 = r0 (bit-trick rsqrt
    # seed), C0 = 1.5, C1 = 0.5. Produces sqrt(x) to ~0.2%.
    _t = Bin(AluOp.MULTIPLY, Src0, Src1)
    nsqrt = mk(
        "NSQRT_ANT",
        Bin(AluOp.MULTIPLY, _t,
            Bin(AluOp.SUBTRACT, C0,
                Bin(AluOp.MULTIPLY, C1,
                    Bin(AluOp.MULTIPLY, _t, Src1)))),
        lambda in0, in1, s0, s1, imm2: (
            (in0.astype(np.float32) * in1.astype(np.float32))
            * (s0 - s1 * (in0.astype(np.float32) * in1.astype(np.float32))
               * in1.astype(np.float32))),
    )
    return addsq, sqacc, nsqrt


def c_lo(i_blk):
    return 896 - 128 * i_blk


@functools.cache
def build_module():
    import concourse.tile_utils as tile_utils
    if getattr(tile_utils, "max_sbuf_usage", 0) < 208 * 1024:
        tile_utils.max_sbuf_usage = 208 * 1024

    # Pin the ACT engine to the ln+exp table set: every activation this
    # kernel emits (Ln, Exp, Copy/Identity) lives in that one set, so the
    # greedy table-load pass emits exactly one LoadActFuncSet.
    import concourse.bacc as bacc_mod
    if not getattr(bacc_mod, "_ant_act_tables_pinned", False):
        orig_gat = bacc_mod.get_activation_tables

        def pinned_gat(arch):
            full = orig_gat(arch)
            out = {}
            for name, funcs in full.items():
                if name != "natural_log_exp_and_others":
                    funcs = funcs - {mybir.ActivationFunctionType.Ln,
                                     mybir.ActivationFunctionType.Exp}
                out[name] = funcs
            return out

        bacc_mod.get_activation_tables = pinned_gat
        bacc_mod._ant_act_tables_pinned = True

    addsq, sqacc, nsqrt = register_custom()
    nc = bacc.Bacc("TRN2", target_bir_lowering=False, debug=False,
                   num_devices=8)

    din = {}
    for nm, shape, dt_ in [
        ("xt_r", [DIM, N], F16), ("xt_i", [DIM, N], F16),
        ("wq_a", [DIM, 512], F16), ("wq_b", [DIM, 512], F16),
        ("wk_a", [DIM, 512], F16), ("wk_b", [DIM, 512], F16),
        ("wv_a", [DIM, 512], F16), ("wv_b", [DIM, 512], F16),
        ("wo_sr", [DIM, 512], F16), ("wo_si", [DIM, 512], F16),
        ("rel_r", [128, 2048], F16), ("rel_i", [128, 2048], F16),
        ("bo_rt", [128, 4], F32), ("bo_it", [128, 4], F32),
        ("smask", [128, 1], F32),
    ]:
        din[nm] = nc.dram_tensor(nm, shape, dt_, kind="ExternalInput")
    o_r = nc.dram_tensor("o_r", [DIM, N], F32, kind="ExternalOutput")
    o_i = nc.dram_tensor("o_i", [DIM, N], F32, kind="ExternalOutput")

    with tile.TileContext(nc) as tc:
        with (
            tc.tile_pool(name="const", bufs=1) as cpool,
            tc.tile_pool(name="wts", bufs=6) as pwt,      # 4KB (weights)
            tc.tile_pool(name="xts", bufs=8) as pxt,      # 2KB (x tiles)
            tc.tile_pool(name="work", bufs=28) as pw,     # 2KB slots
            tc.tile_pool(name="qev", bufs=4) as pqe,      # 4.5KB slots
            tc.tile_pool(name="skew", bufs=6) as psk,     # 4KB slots
            tc.tile_pool(name="stacks", bufs=16) as pstk,  # 2KB slots
            tc.tile_pool(name="vstk", bufs=2) as pvp,     # 4KB slots
            tc.tile_pool(name="outsb", bufs=6) as pout,
            tc.tile_pool(name="small", bufs=16) as psm,
            tc.tile_pool(name="psD", bufs=3, space="PSUM") as psD,   # 2-bank
            tc.tile_pool(name="psC", bufs=2, space="PSUM") as psC,   # 1-bank
            tc.tile_pool(name="dram", bufs=7, space="DRAM") as pdram,
        ):
            # ---------------- constants ----------------
            id16 = cpool.tile([128, 128], F16, tag="id16")
            make_identity(nc, id16[:])
            smask = cpool.tile([128, 1], F32, tag="smask")
            nc.sync.dma_start(smask[:], din["smask"][:, :])

            def load_w4(nm):
                # [512, 512] dram -> [128, 4, 512] tile (one DMA)
                t = pwt.tile([128, 4, 512], F16, tag="w4", name=nm)
                nc.sync.dma_start(
                    t[:],
                    bass.AP(din[nm], 0, [[512, 128], [128 * 512, 4], [1, 512]]))
                return t


            # ---------------- phase P: projections ----------------
            A = [None] * HPC
            A2 = [None] * HPC
            Knat = [None] * HPC

            wq = (load_w4("wq_a"), load_w4("wq_b"))

            # xt tiles: [128, 1024] per (r/i, kt)
            xt = {}
            for nm in ("xt_r", "xt_i"):
                for kt in range(KT):
                    t = pxt.tile([128, 1024], F16, tag="xt",
                                 name=f"{nm}_{kt}")
                    nc.sync.dma_start(
                        t[:], bass.AP(din[nm], kt * 128 * N,
                                      [[N, 128], [1, 1024]]))
                    xt[(nm, kt)] = t

            wk = (load_w4("wk_a"), load_w4("wk_b"))
            wva = load_w4("wv_a")
            wvb = load_w4("wv_b")

            rel_r = cpool.tile([128, 2048], F16, tag="rel_r")
            rel_i = cpool.tile([128, 2048], F16, tag="rel_i")
            nc.sync.dma_start(rel_r[:], din["rel_r"][:, :])
            nc.sync.dma_start(rel_i[:], din["rel_i"][:, :])
            bo_rt = cpool.tile([128, 4], F32, tag="bo_rt")
            bo_it = cpool.tile([128, 4], F32, tag="bo_it")
            nc.sync.dma_start(bo_rt[:], din["bo_rt"][:, :])
            nc.sync.dma_start(bo_it[:], din["bo_it"][:, :])
            wo_sr = cpool.tile([128, 4, 512], F16, tag="wo_sr")
            wo_si = cpool.tile([128, 4, 512], F16, tag="wo_si")
            nc.sync.dma_start(
                wo_sr[:],
                bass.AP(din["wo_sr"], 0, [[512, 128], [128 * 512, 4], [1, 512]]))
            nc.sync.dma_start(
                wo_si[:],
                bass.AP(din["wo_si"], 0, [[512, 128], [128 * 512, 4], [1, 512]]))

            def emit_proj(kind, h, nh):
                wa, wb = wq if kind == "q" else wk
                hs = slice(h * 128, (h + 1) * 128)
                if nh == 0:
                    if kind == "q":
                        A[h] = pstk.tile([128, 1024], F16, tag="stk",
                                         name=f"A{h}")
                    else:
                        Knat[h] = pstk.tile([128, 1024], F16, tag="stk",
                                            name=f"Knat{h}")
                ns = slice(nh * 512, (nh + 1) * 512)
                ps = psD.tile([128, 512], F32, tag="pd",
                              name=f"ps{kind}_{h}_{nh}")
                for kt in range(KT):
                    nc.tensor.matmul(ps[:], wa[:, kt, hs],
                                     xt[("xt_r", kt)][:, ns],
                                     start=(kt == 0), stop=False)
                for kt in range(KT):
                    nc.tensor.matmul(ps[:], wb[:, kt, hs],
                                     xt[("xt_i", kt)][:, ns],
                                     start=False, stop=(kt == KT - 1))
                if kind == "q":
                    nc.scalar.mul(A[h][:, ns], ps[:], smask[:])
                else:
                    nc.scalar.copy(Knat[h][:, ns], ps[:])
                if kind == "q" and nh == 1:
                    # A2 = [s*qi; s*qr] built from A = [s*qr; -s*qi] on
                    # Pool (SBUF->SBUF), so dots_i = A2^T @ Knat and the
                    # Kni2 stack is never materialized
                    A2[h] = pstk.tile([128, 1024], F16, tag="stk",
                                      name=f"A2_{h}")
                    nc.gpsimd.tensor_scalar_mul(A2[h][0:64, :],
                                                A[h][64:128, :], -1.0)
                    nc.gpsimd.tensor_copy(A2[h][64:128, :], A[h][0:64, :])

            for kind, h in (("q", 0), ("q", 1), ("k", 0)):
                for nh in range(2):
                    emit_proj(kind, h, nh)
            Vpp = [pvp.tile([128, 8, 256], F16, tag="vs", name=f"Vpp{p}")
                   for p in range(2)]

            def emit_vproj(p, J):
                # two heads per matmul (256-wide output)
                hs = slice(p * 256, (p + 1) * 256)
                js = slice(J * 128, (J + 1) * 128)
                vps = psC.tile([128, 256], F32, tag="pc",
                               name=f"vps_{p}_{J}")
                for kt in range(KT):
                    nc.tensor.matmul(vps[:],
                                     xt[("xt_r", kt)][:, js],
                                     wva[:, kt, hs],
                                     start=(kt == 0), stop=False)
                for kt in range(KT):
                    nc.tensor.matmul(vps[:],
                                     xt[("xt_i", kt)][:, js],
                                     wvb[:, kt, hs],
                                     start=False, stop=(kt == KT - 1))
                nc.scalar.copy(Vpp[p][:, J, :], vps[:])

            units = ([("k", 1, 0), ("k", 1, 1)]
                     + [("v", 0, J) for J in range(NT)]
                     + [(k, h, nh) for h in (2, 3) for k in ("q", "k")
                        for nh in range(2)]
                     + [("v", 1, J) for J in range(NT)])

            # OT stacks for the output projection:
            # OT[h]: partitions 0:64 = or-dims of head h, 64:128 =
            # oi-dims -- avs's native layout, so stage_D needs ONE copy;
            # the to_out weights are host-packed in matching per-head
            # [wo_r_h; -wo_i_h] row chunks
            OT = [pstk.tile([128, 1024], F16, tag="stk", name=f"OT{t}")
                  for t in range(4)]

            # ---------------- phase A: pipelined attention ----------------
            # Per (h, I): qrel r-part at step s, i-part + skew write/read at
            # step s+1, dots + softmax at s+PB, transpose at s+PB+2, AV at
            # s+PB+3.
            def emit_qrel_r(h, I):
                isl = slice(I * 128, (I + 1) * 128)
                lo = c_lo(I)
                qe = pqe.tile([128, 2, 1152], F16, tag="qe",
                              name=f"qe_{h}_{I}")
                qtl = psC.tile([128, 2, 128], F32, tag="pc",
                               name=f"qtl_{h}_{I}")
                qps = psD.tile([128, 1024], F32, tag="pd",
                               name=f"qpsr_{h}_{I}")
                for c0 in (0, 512):
                    nc.tensor.matmul(qps[:, c0:c0 + 512], A[h][:, isl],
                                     rel_r[:, lo + c0:lo + c0 + 512],
                                     start=True, stop=True)
                nc.tensor.matmul(qtl[:, 0, :], A[h][:, isl],
                                 rel_r[:, lo + 1024:lo + 1152],
                                 start=True, stop=True)
                nc.scalar.copy(qe[:, 0, 0:1024], qps[:])
                return {"qe": qe, "qtl": qtl}

            def emit_qrel_i(h, I, st):
                isl = slice(I * 128, (I + 1) * 128)
                lo = c_lo(I)
                qe, qtl = st["qe"], st["qtl"]
                qps = psD.tile([128, 1024], F32, tag="pd",
                               name=f"qpsi_{h}_{I}")
                for c0 in (0, 512):
                    nc.tensor.matmul(qps[:, c0:c0 + 512], A[h][:, isl],
                                     rel_i[:, lo + c0:lo + c0 + 512],
                                     start=True, stop=True)
                nc.tensor.matmul(qtl[:, 1, :], A[h][:, isl],
                                 rel_i[:, lo + 1024:lo + 1152],
                                 start=True, stop=True)
                nc.scalar.copy(qe[:, 1, 0:1024], qps[:])
                nc.scalar.copy(qe[:, :, 1024:1152], qtl[:])
                slot = pdram.tile([128, 2304], F16, tag="qrev",
                                  name=f"qrev_{h}_{I}")
                nc.sync.dma_start(slot[:, :], qe[:])
                skw = psk.tile([128, 2048], F16, tag="skw",
                               name=f"skew_{h}_{I}")
                nc.sync.dma_start(
                    skw[:],
                    bass.AP(slot.tensor, 127,
                            [[2303, 128], [1152, 2], [1, 1024]]))
                return skw

            def stage_B(h, I, skw):
                isl = slice(I * 128, (I + 1) * 128)
                dpsr = psD.tile([128, 1024], F32, tag="pd",
                                name=f"dpsr_{h}_{I}")
                dpsi = psD.tile([128, 1024], F32, tag="pd",
                                name=f"dpsi_{h}_{I}")
                # i-part skew injected into the dots_i PSUM group via
                # identity-matmul accumulation (DVE can read only one PSUM
                # operand per op, so r-part adds via the fused ADDSQ)
                for nh in range(2):
                    ns = slice(nh * 512, (nh + 1) * 512)
                    nc.tensor.matmul(dpsr[:, ns], A[h][:, isl],
                                     Knat[h][:, ns], start=True, stop=True)
                    nc.tensor.matmul(dpsi[:, ns], A2[h][:, isl],
                                     Knat[h][:, ns], start=True, stop=False)
                    nc.tensor.matmul(dpsi[:, ns], id16[:],
                                     skw[:, 1024 + nh * 512:1024 + nh * 512 + 512],
                                     start=False, stop=True)
                er = pw.tile([128, 1024], F16, tag="wk", name=f"er_{h}_{I}")
                nc.vector._custom_dve(addsq, out=er[:],
                                      in0=skw[:, 0:1024], in1=dpsr[:])
                m2 = pw.tile([128, 1024], F16, tag="wk", name=f"m2_{h}_{I}")
                nc.vector._custom_dve(sqacc, out=m2[:],
                                      in0=er[:], in1=dpsi[:])
                # sqrt(m2) without the ACT ln/exp round trip: integer-view
                # rsqrt bit-seed on Pool, then one fused Newton step * m2 on
                # DVE (NSQRT, next pipeline step).  bits(r0) = 22971 -
                # bits(m2)/2.
                r0 = pw.tile([128, 1024], F16, tag="wk", name=f"r0_{h}_{I}")
                nc.gpsimd.tensor_scalar(
                    r0[:].bitcast(U16), m2[:].bitcast(U16),
                    -0.5, 22971.0, ALU.mult, ALU.add)
                return {"m2": m2, "r0": r0}

            def stage_B2(h, I, st):
                m2, r0 = st["m2"], st["r0"]
                mt = pw.tile([128, 1024], F16, tag="wk", name=f"mt_{h}_{I}")
                nc.vector._custom_dve(nsqrt, out=mt[:],
                                      in0=m2[:], in1=r0[:],
                                      s0=1.5, s1=0.5)
                attn = pw.tile([128, 1024], F16, tag="wk",
                               name=f"attn_{h}_{I}")
                rs = psm.tile([128, 1], F32, tag="sm", name=f"rs_{h}_{I}")
                nc.scalar.activation(attn[:], mt[:], AF.Exp, accum_out=rs[:])
                rc = psm.tile([128, 1], F32, tag="sm", name=f"rc_{h}_{I}")
                nc.vector.reciprocal(rc[:], rs[:])
                # normalize on Pool (SBUF-only op), a full step ahead of the
                # transposes that consume attn
                nc.gpsimd.tensor_scalar_mul(attn[:], attn[:], rc[:])
                return {"attn": attn}

            def stage_C(h, I, st):
                attn = st["attn"]
                tps = psC.tile([128, 1024], F16, tag="pc", name=f"tps_{h}_{I}")
                for J in range(NT):
                    js = slice(J * 128, (J + 1) * 128)
                    nc.tensor.transpose(tps[:, js], attn[:, js], id16[:])
                atT = pw.tile([128, 1024], F16, tag="wk", name=f"atT_{h}_{I}")
                nc.vector.tensor_copy(atT[:], tps[:])
                return atT

            def stage_D(h, I, atT):
                isl = slice(I * 128, (I + 1) * 128)
                avs = psC.tile([128, 128], F32, tag="pc", name=f"avs_{h}_{I}")
                vsl = slice((h % 2) * 128, (h % 2) * 128 + 128)
                for J in range(NT):
                    js = slice(J * 128, (J + 1) * 128)
                    nc.tensor.matmul(avs[:], Vpp[h // 2][:, J, vsl],
                                     atT[:, js],
                                     start=(J == 0), stop=(J == NT - 1))
                nc.vector.tensor_copy(OT[h][:, isl], avs[:])

            def emit_outproj_unit(nh, part, dt_):
                ns = slice(nh * 512, (nh + 1) * 512)
                wo_t, bo_t = (wo_sr, bo_rt) if part == "r" else (wo_si, bo_it)
                if True:
                    if True:
                        ds = slice(dt_ * 128, (dt_ + 1) * 128)
                        ops = psC.tile([128, 512], F32, tag="pc",
                                       name=f"ops_{part}_{dt_}_{nh}")
                        for j in range(4):
                            nc.tensor.matmul(ops[:], wo_t[:, j, ds],
                                             OT[j][:, ns],
                                             start=(j == 0), stop=(j == 3))
                        osb = pout.tile([128, 512], F32, tag="ot",
                                        name=f"osb_{part}_{dt_}_{nh}")
                        if nh == 1:
                            # drain tail: DVE empties ~16us before ACT here
                            nc.vector.tensor_scalar_add(osb[:], ops[:],
                                                        bo_t[:, dt_:dt_ + 1])
                        else:
                            nc.scalar.activation(osb[:], ops[:], AF.Identity,
                                                 bias=bo_t[:, dt_:dt_ + 1])
                        dst = o_r if part == "r" else o_i
                        nc.sync.dma_start(
                            bass.AP(dst, dt_ * 128 * N + nh * 512,
                                    [[N, 128], [1, 512]]),
                            osb[:])

            def emit_outproj(nh):
                for part in ("r", "i"):
                    for dt_ in range(4):
                        emit_outproj_unit(nh, part, dt_)

            ounits = []

            flat = [(h, I) for h in range(HPC) for I in range(NT)]
            PB2, PC, PD = PB + 1, PB + 2, PB + 3
            rmap, skewmap, bmap, b2map, cmap = {}, {}, {}, {}, {}
            # late stages are emitted FIRST within each step so that
            # dependency waits of the stage-B tail never head-of-line block
            # the in-order engine queues for already-runnable work
            for s in range(len(flat) + PD + 1):
                for _ in range(2):
                    if units:
                        u = units.pop(0)
                        if u[0] == "v":
                            emit_vproj(u[1], u[2])
                        else:
                            emit_proj(*u)
                if s < len(flat):
                    h, I = flat[s]
                    rmap[(h, I)] = emit_qrel_r(h, I)
                if 1 <= s < len(flat) + 1:
                    h, I = flat[s - 1]
                    skewmap[(h, I)] = emit_qrel_i(h, I, rmap.pop((h, I)))
                if PB <= s < len(flat) + PB:
                    h, I = flat[s - PB]
                    bmap[(h, I)] = stage_B(h, I, skewmap.pop((h, I)))
                if PB2 <= s < len(flat) + PB2:
                    h, I = flat[s - PB2]
                    b2map[(h, I)] = stage_B2(h, I, bmap.pop((h, I)))
                if PC <= s < len(flat) + PC:
                    h, I = flat[s - PC]
                    cmap[(h, I)] = stage_C(h, I, b2map.pop((h, I)))
                for _ in range(3):
                    if ounits:
                        emit_outproj_unit(*ounits.pop(0))
                if PD <= s < len(flat) + PD:
                    h, I = flat[s - PD]
                    stage_D(h, I, cmap.pop((h, I)))
                    if (h, I) == (HPC - 1, 3):
                        ounits.extend((0, part, dt_) for part in ("r", "i")
                                      for dt_ in range(4))
            while ounits:
                emit_outproj_unit(*ounits.pop(0))
            emit_outproj(1)

    nc.compile()
    return nc, addsq


def _prep_core_inputs(inputs, core):
    b, half = core // 2, core % 2
    x = inputs["x"]
    f16 = np.float16
    f32 = np.float32
    xt_r = np.ascontiguousarray(x[b, :, :, 0].T).astype(f16)
    xt_i = np.ascontiguousarray(x[b, :, :, 1].T).astype(f16)

    def pack_ab(wr, wi):
        a = np.empty((DIM, 512), f32)
        bb = np.empty((DIM, 512), f32)
        for hl in range(HPC):
            gh = half * HPC + hl
            cs = slice(gh * DH, (gh + 1) * DH)
            a[:, hl * 128:hl * 128 + 64] = wr[:, cs]
            a[:, hl * 128 + 64:hl * 128 + 128] = wi[:, cs]
            bb[:, hl * 128:hl * 128 + 64] = -wi[:, cs]
            bb[:, hl * 128 + 64:hl * 128 + 128] = wr[:, cs]
        return a.astype(f16), bb.astype(f16)

    wq_a, wq_b = pack_ab(inputs["wq_r"], inputs["wq_i"])
    wk_a, wk_b = pack_ab(inputs["wkv_r"][:, :512], inputs["wkv_i"][:, :512])
    wv_a, wv_b = pack_ab(inputs["wkv_r"][:, 512:], inputs["wkv_i"][:, 512:])

    rs = slice(half * 256, (half + 1) * 256)
    wr, wi = inputs["wo_r"][rs, :], inputs["wo_i"][rs, :]
    wo_sr = np.concatenate(
        [np.concatenate([wr[h * 64:(h + 1) * 64], -wi[h * 64:(h + 1) * 64]],
                        0) for h in range(HPC)], 0).astype(f16)
    wo_si = np.concatenate(
        [np.concatenate([wi[h * 64:(h + 1) * 64], wr[h * 64:(h + 1) * 64]],
                        0) for h in range(HPC)], 0).astype(f16)

    e = np.arange(2047)
    t_ext = inputs["rel_emb"][np.clip(e - 1023, -MAX_POS, MAX_POS) + MAX_POS]
    relrev = t_ext[::-1].astype(f32)           # [2047, 64]
    rel_r = np.zeros((128, 2048), f32)
    rel_i = np.zeros((128, 2048), f32)
    rel_r[0:64, 0:2047] = relrev.T
    rel_i[64:128, 0:2047] = -relrev.T

    bscale = 1.0 if half == 0 else 0.0
    bo_rt = np.ascontiguousarray(
        inputs["bo_r"].reshape(4, 128).T * bscale).astype(f32)
    bo_it = np.ascontiguousarray(
        inputs["bo_i"].reshape(4, 128).T * bscale).astype(f32)
    smask = np.concatenate(
        [np.full(64, SCALE, f32), np.full(64, -SCALE, f32)]).reshape(128, 1)

    return {
        "xt_r": xt_r, "xt_i": xt_i,
        "wq_a": wq_a, "wq_b": wq_b, "wk_a": wk_a, "wk_b": wk_b,
        "wv_a": wv_a, "wv_b": wv_b,
        "wo_sr": wo_sr, "wo_si": wo_si,
        "rel_r": rel_r.astype(f16), "rel_i": rel_i.astype(f16),
        "bo_rt": bo_rt, "bo_it": bo_it, "smask": smask,
    }


_last_results = {}


def kernel(**inputs):
    inputs = {k: np.asarray(v) for k, v in inputs.items()}
    nc, _ = build_module()
    in_maps = [_prep_core_inputs(inputs, c) for c in range(8)]
    res = run_bass_kernel_spmd(nc, in_maps, core_ids=list(range(8)))
    _last_results["res"] = res

    out = np.empty((B, N, DIM, 2), np.float32)
    for b in range(B):
        r = res.results[2 * b]["o_r"] + res.results[2 * b + 1]["o_r"]
        i = res.results[2 * b]["o_i"] + res.results[2 * b + 1]["o_i"]
        out[b, :, :, 0] = r.T
        out[b, :, :, 1] = i.T
    return out
